# revision 3
# baseline (speedup 1.0000x reference)
"""ConcatAttention (Bahdanau-style) Trainium2 kernel.

score = v^T tanh(W [h; s]);  softmax over masked sequence;  out = attn @ h.

Sharding: data-parallel over batch B=32 across 8 NeuronCores (4 batches/core).
Per core: enc_proj matmul in bf16 ([4*2048, 1024] x [1024, 1024]), tanh with
the decoder-projection fused as ACT bias, v-contraction as m=1 matmuls into a
[1,512] PSUM accumulator, masked softmax on [1, 2048], and the attention-
weighted sum as attn-stationary bf16 matmuls over the original-layout copy.
"""

import numpy as np
import ml_dtypes

BF16 = ml_dtypes.bfloat16

B, S, E, H = 32, 2048, 1024, 1024
D = 1024  # decoder dim (IN_FEATURES - E)
N_CORES = 8
BL = B // N_CORES  # batches per core
MASK_VAL = -50000.0

ST = 4        # s tiles of 512 per batch (step A)
SW = S // ST  # 512
HT = 8        # h tiles of 128
EC = 8        # e (contraction) chunks of 128
SC = 16       # s chunks of 128 (step B)
EH = 2        # e halves of 512 (step B)

_CACHE = {}


def _emit(tc, mybir, encT, enc, wencT, wdecT, decT, vT, maskb, out_d, attn_d):
    import concourse.bass as bass  # noqa: F401

    nc = tc.nc
    f32 = mybir.dt.float32
    bf16 = mybir.dt.bfloat16
    Tanh = mybir.ActivationFunctionType.Tanh
    Exp = mybir.ActivationFunctionType.Exp

    from contextlib import ExitStack

    with ExitStack() as ctx:
        consts = ctx.enter_context(tc.tile_pool(name="consts", bufs=1))
        encT_pool = ctx.enter_context(tc.tile_pool(name="encT", bufs=3))
        enc6_pool = ctx.enter_context(tc.tile_pool(name="enc6", bufs=8))
        hid_pool = ctx.enter_context(tc.tile_pool(name="hid", bufs=3))
        lg_pool = ctx.enter_context(tc.tile_pool(name="lg", bufs=2))
        soft_pool = ctx.enter_context(tc.tile_pool(name="soft", bufs=2))
        sc_pool = ctx.enter_context(tc.tile_pool(name="scal", bufs=2))
        pa_pool = ctx.enter_context(tc.tile_pool(name="pa", bufs=2, space="PSUM"))
        pl_pool = ctx.enter_context(tc.tile_pool(name="pl", bufs=2, space="PSUM"))
        pt_pool = ctx.enter_context(tc.tile_pool(name="pt", bufs=2, space="PSUM"))
        po_pool = ctx.enter_context(tc.tile_pool(name="po", bufs=2, space="PSUM"))

        # ---- constants / weights ----
        wenc_sb = consts.tile([128, EC * H], bf16)   # [e%128, e//128 * H + h]
        wdec_sb = consts.tile([128, EC * H], bf16)
        for c in range(EC):
            nc.sync.dma_start(
                out=wenc_sb[:, c * H:(c + 1) * H],
                in_=wencT[c * 128:(c + 1) * 128, :],
            )
            nc.sync.dma_start(
                out=wdec_sb[:, c * H:(c + 1) * H],
                in_=wdecT[c * 128:(c + 1) * 128, :],
            )
        dec_sb = consts.tile([128, EC * BL], bf16)   # [d%128, d//128 * BL + b]
        for c in range(EC):
            nc.sync.dma_start(
                out=dec_sb[:, c * BL:(c + 1) * BL],
                in_=decT[c * 128:(c + 1) * 128, :],
            )
        v_sb = consts.tile([128, HT], bf16)          # v[t*128 + p] at [p, t]
        nc.sync.dma_start(out=v_sb[:], in_=vT[:, :])
        ones_sb = consts.tile([1, 1], f32)
        nc.vector.memset(ones_sb[:], 1.0)
        mask_sb = [consts.tile([1, S], f32, tag=f"mask{b}", name=f"mask{b}") for b in range(BL)]
        for b in range(BL):
            nc.sync.dma_start(out=mask_sb[b][:], in_=maskb[b:b + 1, :])
        # dec projection, laid out transposed: dproj[h%128, (h//128)*BL + b]
        dproj = consts.tile([128, HT * BL], f32)

        # ---- phase 1: dec_proj[h, b] = sum_d W_dec[h, d] dec[b, d] ----
        for t in range(HT):
            psum_d = pa_pool.tile([128, BL], f32, tag="pa")
            for c in range(EC):
                nc.tensor.matmul(
                    psum_d[:],
                    lhsT=wdec_sb[:, c * H + t * 128: c * H + (t + 1) * 128],
                    rhs=dec_sb[:, c * BL:(c + 1) * BL],
                    start=(c == 0),
                    stop=(c == EC - 1),
                )
            nc.scalar.copy(dproj[:, t * BL:(t + 1) * BL], psum_d[:])

        # ---- phase 2, software-pipelined over batches ----
        logits = [None] * BL

        def step_a(b):
            """enc_proj -> tanh -> v-contraction -> masked logits for batch b."""
            logits[b] = lg_pool.tile([1, S], f32, tag="logits", name=f"logits{b}")
            for st in range(ST):
                et = encT_pool.tile([128, EC * SW], bf16, tag="encT")
                for c in range(EC):
                    nc.sync.dma_start(
                        out=et[:, c * SW:(c + 1) * SW],
                        in_=encT[b, c * 128:(c + 1) * 128, st * SW:(st + 1) * SW],
                    )
                psum_l = pl_pool.tile([1, SW], f32, tag="pl")
                for t in range(HT):
                    psum_a = pa_pool.tile([128, SW], f32, tag="pa")
                    for c in range(EC):
                        nc.tensor.matmul(
                            psum_a[:],
                            lhsT=wenc_sb[:, c * H + t * 128: c * H + (t + 1) * 128],
                            rhs=et[:, c * SW:(c + 1) * SW],
                            start=(c == 0),
                            stop=(c == EC - 1),
                        )
                    hid = hid_pool.tile([128, SW], bf16, tag="hid")
                    nc.scalar.activation(
                        hid[:], psum_a[:], Tanh,
                        bias=dproj[:, t * BL + b: t * BL + b + 1],
                    )
                    nc.tensor.matmul(
                        psum_l[:],
                        lhsT=v_sb[:, t:t + 1],
                        rhs=hid[:],
                        start=(t == 0),
                        stop=(t == HT - 1),
                    )
                # logits <- psum_l + mask bias
                nc.vector.tensor_add(
                    logits[b][:, st * SW:(st + 1) * SW],
                    psum_l[:],
                    mask_sb[b][:, st * SW:(st + 1) * SW],
                )

        def soft6(b):
            """softmax + attn output + weighted sum for batch b."""
            mx = sc_pool.tile([1, 1], f32, tag="mx")
            nc.vector.reduce_max(mx[:], logits[b][:], axis=mybir.AxisListType.X)
            negmx = sc_pool.tile([1, 1], f32, tag="negmx")
            nc.vector.tensor_scalar_mul(negmx[:], mx[:], -1.0)
            expu = soft_pool.tile([1, S], f32, tag="expu")
            z = sc_pool.tile([1, 1], f32, tag="z")
            nc.scalar.activation(
                expu[:], logits[b][:], Exp, bias=negmx[:], accum_out=z[:],
            )
            rz = sc_pool.tile([1, 1], f32, tag="rz")
            nc.vector.reciprocal(rz[:], z[:])
            attn = soft_pool.tile([1, S], f32, tag="attn")
            nc.vector.tensor_scalar_mul(attn[:], expu[:], rz[:])
            nc.sync.dma_start(out=attn_d[b:b + 1, :], in_=attn[:])
            # transpose attn to [s%128, s//128] via 16 k=1 matmuls
            psum_t = pt_pool.tile([128, SC], f32, tag="pt")
            for sc in range(SC):
                nc.tensor.matmul(
                    psum_t[:, sc:sc + 1],
                    lhsT=attn[:, sc * 128:(sc + 1) * 128],
                    rhs=ones_sb[:],
                    start=True,
                    stop=True,
                )
            attnT = soft_pool.tile([128, SC], bf16, tag="attnT")
            nc.vector.tensor_copy(attnT[:], psum_t[:])
            # out[e] = sum_s attn[s] enc[s, e]
            psum_o = [po_pool.tile([1, 512], f32, tag="po", name=f"po{eh_}") for eh_ in range(EH)]
            for sc in range(SC):
                e6 = enc6_pool.tile([128, E], bf16, tag="enc6")
                nc.sync.dma_start(
                    out=e6[:],
                    in_=enc[b, sc * 128:(sc + 1) * 128, :],
                )
                for eh in range(EH):
                    nc.tensor.matmul(
                        psum_o[eh][:],
                        lhsT=attnT[:, sc:sc + 1],
                        rhs=e6[:, eh * 512:(eh + 1) * 512],
                        start=(sc == 0),
                        stop=(sc == SC - 1),
                    )
            out_sb = soft_pool.tile([1, E], f32, tag="out_sb")
            for eh in range(EH):
                nc.scalar.copy(out_sb[:, eh * 512:(eh + 1) * 512], psum_o[eh][:])
            nc.sync.dma_start(out=out_d[b:b + 1, :], in_=out_sb[:])

        # pipeline: keep PE busy with batch b+1's matmuls while batch b's
        # softmax chain (DVE/ACT) completes
        step_a(0)
        for b in range(BL):
            if b + 1 < BL:
                step_a(b + 1)
            soft6(b)


def _build():
    import concourse.bacc as bacc
    import concourse.tile as tile
    from concourse import mybir

    f32 = mybir.dt.float32
    bf16 = mybir.dt.bfloat16

    nc = bacc.Bacc("TRN2", target_bir_lowering=False, debug=False,
                   num_devices=N_CORES)
    encT = nc.dram_tensor("encT", [BL, E, S], bf16, kind="ExternalInput").ap()
    enc = nc.dram_tensor("enc", [BL, S, E], bf16, kind="ExternalInput").ap()
    wencT = nc.dram_tensor("wencT", [E, H], bf16, kind="ExternalInput").ap()
    wdecT = nc.dram_tensor("wdecT", [D, H], bf16, kind="ExternalInput").ap()
    decT = nc.dram_tensor("decT", [D, BL], bf16, kind="ExternalInput").ap()
    vT = nc.dram_tensor("vT", [128, HT], bf16, kind="ExternalInput").ap()
    maskb = nc.dram_tensor("maskb", [BL, S], f32, kind="ExternalInput").ap()
    out_d = nc.dram_tensor("out", [BL, E], f32, kind="ExternalOutput").ap()
    attn_d = nc.dram_tensor("attnw", [BL, S], f32, kind="ExternalOutput").ap()

    with tile.TileContext(nc) as tc:
        _emit(tc, mybir, encT, enc, wencT, wdecT, decT, vT, maskb, out_d, attn_d)
    nc.compile()
    return nc


def get_compiled():
    if "nc" not in _CACHE:
        _CACHE["nc"] = _build()
    return _CACHE["nc"]


def make_in_maps(encoder_outputs, sequence_length, decoder_state, W, v):
    encoder_outputs = np.asarray(encoder_outputs, dtype=np.float32)
    sequence_length = np.asarray(sequence_length)
    decoder_state = np.asarray(decoder_state, dtype=np.float32)
    W = np.asarray(W, dtype=np.float32)
    v = np.asarray(v, dtype=np.float32)

    wencT = np.ascontiguousarray(W[:, :E].T).astype(BF16)    # [e, h]
    wdecT = np.ascontiguousarray(W[:, E:].T).astype(BF16)    # [d, h]
    vT = np.ascontiguousarray(v.reshape(HT, 128).T).astype(BF16)  # [p, t]
    mask = np.where(
        np.arange(S)[None, :] < sequence_length[:, None].astype(np.int64),
        np.float32(0.0), np.float32(MASK_VAL),
    ).astype(np.float32)                                     # [B, S]
    decT_full = np.ascontiguousarray(decoder_state.T).astype(BF16)  # [d, B]

    in_maps = []
    for cid in range(N_CORES):
        sl = slice(cid * BL, (cid + 1) * BL)
        enc_c = encoder_outputs[sl]
        in_maps.append({
            "encT": np.ascontiguousarray(enc_c.transpose(0, 2, 1)).astype(BF16),
            "enc": enc_c.astype(BF16),
            "wencT": wencT,
            "wdecT": wdecT,
            "decT": np.ascontiguousarray(decT_full[:, sl]),
            "vT": vT,
            "maskb": np.ascontiguousarray(mask[sl]),
        })
    return in_maps


def run(in_maps, trace=False, **kw):
    from concourse.bass_utils import run_bass_kernel_spmd

    nc = get_compiled()
    return run_bass_kernel_spmd(nc, in_maps, list(range(N_CORES)), trace=trace, **kw)


def kernel(encoder_outputs, sequence_length, decoder_state, W, v):
    in_maps = make_in_maps(encoder_outputs, sequence_length, decoder_state, W, v)
    res = run(in_maps)
    out = np.concatenate(
        [np.asarray(res.results[i]["out"], dtype=np.float32) for i in range(N_CORES)], axis=0)
    attn = np.concatenate(
        [np.asarray(res.results[i]["attnw"], dtype=np.float32) for i in range(N_CORES)], axis=0)
    return out, attn


# revision 4
# speedup vs baseline: 1.1451x; 1.1451x over previous
"""ConcatAttention (Bahdanau-style) Trainium2 kernel.

score = v^T tanh(W [h; s]);  softmax over masked sequence;  out = attn @ h.

Sharding: data-parallel over batch B=32 across 8 NeuronCores (4 batches/core).
Per core: enc_proj matmul in bf16 ([4*2048, 1024] x [1024, 1024]), tanh with
the decoder-projection fused as ACT bias; the v-contraction runs on the Vector
engine as a scalar*tensor+tensor accumulate chain, finished by a ones-vector
matmul for the cross-partition sum; masked softmax on [1, 2048]; the
attention-weighted sum uses unnormalized exp weights (attn-stationary bf16
matmuls over the original-layout copy) with the 1/Z scale fused into the
PSUM eviction.
"""

import numpy as np
import ml_dtypes

BF16 = ml_dtypes.bfloat16

B, S, E, H = 32, 2048, 1024, 1024
D = 1024  # decoder dim (IN_FEATURES - E)
N_CORES = 8
BL = B // N_CORES  # batches per core
MASK_VAL = -50000.0

ST = 4        # s tiles of 512 per batch (step A)
SW = S // ST  # 512
HT = 8        # h tiles of 128
EC = 8        # e (contraction) chunks of 128
SC = 16       # s chunks of 128 (step B)
EH = 2        # e halves of 512 (step B)

_CACHE = {}


def _emit(tc, mybir, encT, enc, wencT, wdecT, decT, vTf, maskb, out_d, attn_d):
    nc = tc.nc
    f32 = mybir.dt.float32
    bf16 = mybir.dt.bfloat16
    Tanh = mybir.ActivationFunctionType.Tanh
    Exp = mybir.ActivationFunctionType.Exp
    Copy = mybir.ActivationFunctionType.Copy
    AX = mybir.AxisListType.X
    mult = mybir.AluOpType.mult
    add = mybir.AluOpType.add

    from contextlib import ExitStack

    with ExitStack() as ctx:
        consts = ctx.enter_context(tc.tile_pool(name="consts", bufs=1))
        encT_pool = ctx.enter_context(tc.tile_pool(name="encT", bufs=3))
        enc6_pool = ctx.enter_context(tc.tile_pool(name="enc6", bufs=8))
        hid_pool = ctx.enter_context(tc.tile_pool(name="hid", bufs=3))
        acc_pool = ctx.enter_context(tc.tile_pool(name="acc", bufs=2))
        lg_pool = ctx.enter_context(tc.tile_pool(name="lg", bufs=2))
        soft_pool = ctx.enter_context(tc.tile_pool(name="soft", bufs=2))
        sc_pool = ctx.enter_context(tc.tile_pool(name="scal", bufs=2))
        pa_pool = ctx.enter_context(tc.tile_pool(name="pa", bufs=2, space="PSUM"))
        pl_pool = ctx.enter_context(tc.tile_pool(name="pl", bufs=2, space="PSUM"))
        pt_pool = ctx.enter_context(tc.tile_pool(name="pt", bufs=2, space="PSUM"))
        po_pool = ctx.enter_context(tc.tile_pool(name="po", bufs=2, space="PSUM"))

        # ---- weights for step A on the sync queue (PE's first dependency) ----
        wenc_sb = consts.tile([128, EC * H], bf16)   # [e%128, e//128 * H + h]
        for c in range(EC):
            nc.sync.dma_start(
                out=wenc_sb[:, c * H:(c + 1) * H],
                in_=wencT[c * 128:(c + 1) * 128, :],
            )
        # ---- remaining constants on the gpsimd (SWDGE) queue, in parallel ----
        wdec_sb = consts.tile([128, EC * H], bf16)
        for c in range(EC):
            nc.gpsimd.dma_start(
                out=wdec_sb[:, c * H:(c + 1) * H],
                in_=wdecT[c * 128:(c + 1) * 128, :],
            )
        dec_sb = consts.tile([128, EC * BL], bf16)   # [d%128, d//128 * BL + b]
        for c in range(EC):
            nc.gpsimd.dma_start(
                out=dec_sb[:, c * BL:(c + 1) * BL],
                in_=decT[c * 128:(c + 1) * 128, :],
            )
        vf_sb = consts.tile([128, HT], f32)          # v[t*128 + p] at [p, t]
        nc.gpsimd.dma_start(out=vf_sb[:], in_=vTf[:, :])
        ones_sb = consts.tile([1, 1], f32)
        nc.vector.memset(ones_sb[:], 1.0)
        ones_bf = consts.tile([128, 1], bf16)
        nc.vector.memset(ones_bf[:], 1.0)
        mask_sb = [consts.tile([1, S], f32, tag=f"mask{b}", name=f"mask{b}")
                   for b in range(BL)]
        for b in range(BL):
            nc.gpsimd.dma_start(out=mask_sb[b][:], in_=maskb[b:b + 1, :])
        # dec projection, laid out transposed: dproj[h%128, (h//128)*BL + b]
        dproj = consts.tile([128, HT * BL], f32)

        # ---- phase 1: dec_proj[h, b] = sum_d W_dec[h, d] dec[b, d] ----
        for t in range(HT):
            psum_d = pa_pool.tile([128, BL], f32, tag="pa")
            for c in range(EC):
                nc.tensor.matmul(
                    psum_d[:],
                    lhsT=wdec_sb[:, c * H + t * 128: c * H + (t + 1) * 128],
                    rhs=dec_sb[:, c * BL:(c + 1) * BL],
                    start=(c == 0),
                    stop=(c == EC - 1),
                )
            nc.scalar.copy(dproj[:, t * BL:(t + 1) * BL], psum_d[:])

        # ---- phase 2, software-pipelined over batches ----
        logits = [None] * BL
        mx4 = [None] * BL

        def step_a(b):
            """enc_proj -> tanh -> v-accumulate -> masked logits for batch b."""
            logits[b] = lg_pool.tile([1, S], f32, tag="logits", name=f"logits{b}")
            mx4[b] = lg_pool.tile([1, ST], f32, tag="mx4", name=f"mx4_{b}")
            for st in range(ST):
                et = encT_pool.tile([128, EC * SW], bf16, tag="encT")
                for c in range(EC):
                    nc.sync.dma_start(
                        out=et[:, c * SW:(c + 1) * SW],
                        in_=encT[b, c * 128:(c + 1) * 128, st * SW:(st + 1) * SW],
                    )
                acc = acc_pool.tile([128, SW], f32, tag="acc")
                acc_bf = acc_pool.tile([128, SW], bf16, tag="accbf")
                for t in range(HT):
                    psum_a = pa_pool.tile([128, SW], f32, tag="pa")
                    for c in range(EC):
                        nc.tensor.matmul(
                            psum_a[:],
                            lhsT=wenc_sb[:, c * H + t * 128: c * H + (t + 1) * 128],
                            rhs=et[:, c * SW:(c + 1) * SW],
                            start=(c == 0),
                            stop=(c == EC - 1),
                        )
                    hid = hid_pool.tile([128, SW], bf16, tag="hid")
                    nc.scalar.activation(
                        hid[:], psum_a[:], Tanh,
                        bias=dproj[:, t * BL + b: t * BL + b + 1],
                    )
                    # acc += v_t * tanh(...), on the Vector engine
                    if t == 0:
                        nc.vector.tensor_scalar_mul(acc[:], hid[:], vf_sb[:, 0:1])
                    elif t < HT - 1:
                        nc.vector.scalar_tensor_tensor(
                            acc[:], hid[:], vf_sb[:, t:t + 1], acc[:], mult, add)
                    else:
                        nc.vector.scalar_tensor_tensor(
                            acc_bf[:], hid[:], vf_sb[:, t:t + 1], acc[:], mult, add)
                # cross-partition sum via ones-vector matmul
                psum_l = pl_pool.tile([1, SW], f32, tag="pl")
                nc.tensor.matmul(psum_l[:], lhsT=ones_bf[:], rhs=acc_bf[:],
                                 start=True, stop=True)
                # logits <- psum_l + mask bias; running per-tile max
                nc.vector.tensor_add(
                    logits[b][:, st * SW:(st + 1) * SW],
                    psum_l[:],
                    mask_sb[b][:, st * SW:(st + 1) * SW],
                )
                nc.vector.reduce_max(
                    mx4[b][:, st:st + 1],
                    logits[b][:, st * SW:(st + 1) * SW], axis=AX)

        def soft6(b):
            """softmax + attn output + weighted sum for batch b."""
            mx = sc_pool.tile([1, 1], f32, tag="mx")
            nc.vector.reduce_max(mx[:], mx4[b][:], axis=AX)
            negmx = sc_pool.tile([1, 1], f32, tag="negmx")
            nc.vector.tensor_scalar_mul(negmx[:], mx[:], -1.0)
            expu = soft_pool.tile([1, S], f32, tag="expu")
            z = sc_pool.tile([1, 1], f32, tag="z")
            nc.scalar.activation(
                expu[:], logits[b][:], Exp, bias=negmx[:], accum_out=z[:],
            )
            rz = sc_pool.tile([1, 1], f32, tag="rz")
            nc.vector.reciprocal(rz[:], z[:])
            # normalized attention weights output (off the critical path)
            attn = soft_pool.tile([1, S], f32, tag="attn")
            nc.vector.tensor_scalar_mul(attn[:], expu[:], rz[:])
            nc.sync.dma_start(out=attn_d[b:b + 1, :], in_=attn[:])
            # transpose unnormalized expu to [s%128, s//128] via k=1 matmuls
            psum_t = pt_pool.tile([128, SC], f32, tag="pt")
            for sc in range(SC):
                nc.tensor.matmul(
                    psum_t[:, sc:sc + 1],
                    lhsT=expu[:, sc * 128:(sc + 1) * 128],
                    rhs=ones_sb[:],
                    start=True,
                    stop=True,
                )
            attnT = soft_pool.tile([128, SC], bf16, tag="attnT")
            nc.vector.tensor_copy(attnT[:], psum_t[:])
            # out[e] = (1/Z) sum_s expu[s] enc[s, e]
            psum_o = [po_pool.tile([1, 512], f32, tag="po", name=f"po{eh_}")
                      for eh_ in range(EH)]
            for sc in range(SC):
                e6 = enc6_pool.tile([128, E], bf16, tag="enc6")
                nc.sync.dma_start(
                    out=e6[:],
                    in_=enc[b, sc * 128:(sc + 1) * 128, :],
                )
                for eh in range(EH):
                    nc.tensor.matmul(
                        psum_o[eh][:],
                        lhsT=attnT[:, sc:sc + 1],
                        rhs=e6[:, eh * 512:(eh + 1) * 512],
                        start=(sc == 0),
                        stop=(sc == SC - 1),
                    )
            out_sb = soft_pool.tile([1, E], f32, tag="out_sb")
            for eh in range(EH):
                nc.scalar.activation(
                    out_sb[:, eh * 512:(eh + 1) * 512], psum_o[eh][:],
                    Copy, scale=rz[:])
            nc.sync.dma_start(out=out_d[b:b + 1, :], in_=out_sb[:])

        # pipeline: keep PE busy with batch b+1's matmuls while batch b's
        # softmax chain (DVE/ACT) completes
        step_a(0)
        for b in range(BL):
            if b + 1 < BL:
                step_a(b + 1)
            soft6(b)


def _build():
    import concourse.bacc as bacc
    import concourse.tile as tile
    from concourse import mybir

    f32 = mybir.dt.float32
    bf16 = mybir.dt.bfloat16

    nc = bacc.Bacc("TRN2", target_bir_lowering=False, debug=False,
                   num_devices=N_CORES)
    encT = nc.dram_tensor("encT", [BL, E, S], bf16, kind="ExternalInput").ap()
    enc = nc.dram_tensor("enc", [BL, S, E], bf16, kind="ExternalInput").ap()
    wencT = nc.dram_tensor("wencT", [E, H], bf16, kind="ExternalInput").ap()
    wdecT = nc.dram_tensor("wdecT", [D, H], bf16, kind="ExternalInput").ap()
    decT = nc.dram_tensor("decT", [D, BL], bf16, kind="ExternalInput").ap()
    vTf = nc.dram_tensor("vTf", [128, HT], f32, kind="ExternalInput").ap()
    maskb = nc.dram_tensor("maskb", [BL, S], f32, kind="ExternalInput").ap()
    out_d = nc.dram_tensor("out", [BL, E], f32, kind="ExternalOutput").ap()
    attn_d = nc.dram_tensor("attnw", [BL, S], f32, kind="ExternalOutput").ap()

    with tile.TileContext(nc) as tc:
        _emit(tc, mybir, encT, enc, wencT, wdecT, decT, vTf, maskb, out_d, attn_d)
    nc.compile()
    return nc


def get_compiled():
    if "nc" not in _CACHE:
        _CACHE["nc"] = _build()
    return _CACHE["nc"]


def make_in_maps(encoder_outputs, sequence_length, decoder_state, W, v):
    encoder_outputs = np.asarray(encoder_outputs, dtype=np.float32)
    sequence_length = np.asarray(sequence_length)
    decoder_state = np.asarray(decoder_state, dtype=np.float32)
    W = np.asarray(W, dtype=np.float32)
    v = np.asarray(v, dtype=np.float32)

    wencT = np.ascontiguousarray(W[:, :E].T).astype(BF16)    # [e, h]
    wdecT = np.ascontiguousarray(W[:, E:].T).astype(BF16)    # [d, h]
    vTf = np.ascontiguousarray(v.reshape(HT, 128).T).astype(np.float32)  # [p, t]
    mask = np.where(
        np.arange(S)[None, :] < sequence_length[:, None].astype(np.int64),
        np.float32(0.0), np.float32(MASK_VAL),
    ).astype(np.float32)                                     # [B, S]
    decT_full = np.ascontiguousarray(decoder_state.T).astype(BF16)  # [d, B]

    in_maps = []
    for cid in range(N_CORES):
        sl = slice(cid * BL, (cid + 1) * BL)
        enc_c = encoder_outputs[sl]
        in_maps.append({
            "encT": np.ascontiguousarray(enc_c.transpose(0, 2, 1)).astype(BF16),
            "enc": enc_c.astype(BF16),
            "wencT": wencT,
            "wdecT": wdecT,
            "decT": np.ascontiguousarray(decT_full[:, sl]),
            "vTf": vTf,
            "maskb": np.ascontiguousarray(mask[sl]),
        })
    return in_maps


def run(in_maps, trace=False, **kw):
    from concourse.bass_utils import run_bass_kernel_spmd

    nc = get_compiled()
    return run_bass_kernel_spmd(nc, in_maps, list(range(N_CORES)), trace=trace, **kw)


def kernel(encoder_outputs, sequence_length, decoder_state, W, v):
    in_maps = make_in_maps(encoder_outputs, sequence_length, decoder_state, W, v)
    res = run(in_maps)
    out = np.concatenate(
        [np.asarray(res.results[i]["out"], dtype=np.float32) for i in range(N_CORES)], axis=0)
    attn = np.concatenate(
        [np.asarray(res.results[i]["attnw"], dtype=np.float32) for i in range(N_CORES)], axis=0)
    return out, attn


# revision 6
# speedup vs baseline: 1.1601x; 1.0131x over previous
"""ConcatAttention (Bahdanau-style) Trainium2 kernel.

score = v^T tanh(W [h; s]);  softmax over masked sequence;  out = attn @ h.

Sharding: data-parallel over batch B=32 across 8 NeuronCores (4 batches/core).
Per core: enc_proj matmul in bf16 ([4*2048, 1024] x [1024, 1024]), tanh with
the decoder-projection fused as ACT bias; the v-contraction runs on the Vector
engine as a scalar*tensor+tensor accumulate chain, finished by a ones-vector
matmul for the cross-partition sum; masked softmax on [1, 2048]; the
attention-weighted sum uses unnormalized exp weights (attn-stationary bf16
matmuls over the original-layout copy) with the 1/Z scale fused into the
PSUM eviction.  Heavy DMA streams are round-robined across the sync, scalar,
vector and gpsimd engine queues.
"""

import numpy as np
import ml_dtypes

BF16 = ml_dtypes.bfloat16

B, S, E, H = 32, 2048, 1024, 1024
D = 1024  # decoder dim (IN_FEATURES - E)
N_CORES = 8
BL = B // N_CORES  # batches per core
MASK_VAL = -50000.0

ST = 4        # s tiles of 512 per batch (step A)
SW = S // ST  # 512
HT = 8        # h tiles of 128
EC = 8        # e (contraction) chunks of 128
SC = 16       # s chunks of 128 (step B)
EH = 2        # e halves of 512 (step B)

_CACHE = {}


def _emit(tc, mybir, encT2, enc, wencT, dprojT, vTf, maskb, out_d, attn_d):
    nc = tc.nc
    f32 = mybir.dt.float32
    bf16 = mybir.dt.bfloat16
    Tanh = mybir.ActivationFunctionType.Tanh
    Exp = mybir.ActivationFunctionType.Exp
    Copy = mybir.ActivationFunctionType.Copy
    AX = mybir.AxisListType.X
    mult = mybir.AluOpType.mult
    add = mybir.AluOpType.add

    dmae = [nc.sync, nc.scalar, nc.gpsimd]  # one DMA queue each

    from contextlib import ExitStack

    with ExitStack() as ctx:
        consts = ctx.enter_context(tc.tile_pool(name="consts", bufs=1))
        encT_pool = ctx.enter_context(tc.tile_pool(name="encT", bufs=3))
        enc6_pool = ctx.enter_context(tc.tile_pool(name="enc6", bufs=8))
        hid_pool = ctx.enter_context(tc.tile_pool(name="hid", bufs=3))
        acc_pool = ctx.enter_context(tc.tile_pool(name="acc", bufs=2))
        lg_pool = ctx.enter_context(tc.tile_pool(name="lg", bufs=2))
        soft_pool = ctx.enter_context(tc.tile_pool(name="soft", bufs=2))
        sc_pool = ctx.enter_context(tc.tile_pool(name="scal", bufs=2))
        pa_pool = ctx.enter_context(tc.tile_pool(name="pa", bufs=2, space="PSUM"))
        pl_pool = ctx.enter_context(tc.tile_pool(name="pl", bufs=2, space="PSUM"))
        pt_pool = ctx.enter_context(tc.tile_pool(name="pt", bufs=2, space="PSUM"))
        po_pool = ctx.enter_context(tc.tile_pool(name="po", bufs=2, space="PSUM"))

        # ---- step-A weights, round-robined across all four queues ----
        wenc_sb = consts.tile([128, EC * H], bf16)   # [e%128, e//128 * H + h]
        for c in range(EC):
            dmae[c % 3].dma_start(
                out=wenc_sb[:, c * H:(c + 1) * H],
                in_=wencT[c * 128:(c + 1) * 128, :],
            )
        # small constants on the gpsimd queue
        vf_sb = consts.tile([128, HT], f32)          # v[t*128 + p] at [p, t]
        nc.gpsimd.dma_start(out=vf_sb[:], in_=vTf[:, :])
        # dec projection (host-computed), transposed: [h%128, (h//128)*BL + b]
        dproj = consts.tile([128, HT * BL], f32)
        nc.gpsimd.dma_start(out=dproj[:], in_=dprojT[:, :])
        ones_sb = consts.tile([1, 1], f32)
        nc.vector.memset(ones_sb[:], 1.0)
        ones_bf = consts.tile([128, 1], bf16)
        nc.vector.memset(ones_bf[:], 1.0)
        mask_sb = [consts.tile([1, S], f32, tag=f"mask{b}", name=f"mask{b}")
                   for b in range(BL)]
        for b in range(BL):
            nc.gpsimd.dma_start(out=mask_sb[b][:], in_=maskb[b:b + 1, :])

        logits = [None] * BL
        mx4 = [None] * BL

        def step_a(b):
            """enc_proj -> tanh -> v-accumulate -> masked logits for batch b."""
            logits[b] = lg_pool.tile([1, S], f32, tag="logits", name=f"logits{b}")
            mx4[b] = lg_pool.tile([1, ST], f32, tag="mx4", name=f"mx4_{b}")
            for st in range(ST):
                et = encT_pool.tile([128, EC * SW], bf16, tag="encT")
                # one 0.5 MiB DMA per half, on two different queues
                for half in range(2):
                    dmae[(2 * (b * ST + st) + half) % 3].dma_start(
                        out=et[:, half * 4 * SW:(half + 1) * 4 * SW],
                        in_=encT2[b, :, half * 4:(half + 1) * 4, st, :],
                    )
                acc = acc_pool.tile([128, SW], f32, tag="acc")
                acc_bf = acc_pool.tile([128, SW], bf16, tag="accbf")
                for t in range(HT):
                    psum_a = pa_pool.tile([128, SW], f32, tag="pa")
                    for c in range(EC):
                        nc.tensor.matmul(
                            psum_a[:],
                            lhsT=wenc_sb[:, c * H + t * 128: c * H + (t + 1) * 128],
                            rhs=et[:, c * SW:(c + 1) * SW],
                            start=(c == 0),
                            stop=(c == EC - 1),
                        )
                    hid = hid_pool.tile([128, SW], bf16, tag="hid")
                    nc.scalar.activation(
                        hid[:], psum_a[:], Tanh,
                        bias=dproj[:, t * BL + b: t * BL + b + 1],
                    )
                    # acc += v_t * tanh(...), on the Vector engine
                    if t == 0:
                        nc.vector.tensor_scalar_mul(acc[:], hid[:], vf_sb[:, 0:1])
                    elif t < HT - 1:
                        nc.vector.scalar_tensor_tensor(
                            acc[:], hid[:], vf_sb[:, t:t + 1], acc[:], mult, add)
                    else:
                        nc.vector.scalar_tensor_tensor(
                            acc_bf[:], hid[:], vf_sb[:, t:t + 1], acc[:], mult, add)
                # cross-partition sum via ones-vector matmul
                psum_l = pl_pool.tile([1, SW], f32, tag="pl")
                nc.tensor.matmul(psum_l[:], lhsT=ones_bf[:], rhs=acc_bf[:],
                                 start=True, stop=True)
                # logits <- psum_l + mask bias; running per-tile max
                nc.vector.tensor_add(
                    logits[b][:, st * SW:(st + 1) * SW],
                    psum_l[:],
                    mask_sb[b][:, st * SW:(st + 1) * SW],
                )
                nc.vector.reduce_max(
                    mx4[b][:, st:st + 1],
                    logits[b][:, st * SW:(st + 1) * SW], axis=AX)

        def soft6(b):
            """softmax + attn output + weighted sum for batch b."""
            mx = sc_pool.tile([1, 1], f32, tag="mx")
            nc.vector.reduce_max(mx[:], mx4[b][:], axis=AX)
            negmx = sc_pool.tile([1, 1], f32, tag="negmx")
            nc.vector.tensor_scalar_mul(negmx[:], mx[:], -1.0)
            expu = soft_pool.tile([1, S], f32, tag="expu")
            z4 = sc_pool.tile([1, ST], f32, tag="z4")
            psum_t = pt_pool.tile([128, SC], f32, tag="pt")
            # chunked exp releases the transposes early (shorter tail)
            for st in range(ST):
                nc.scalar.activation(
                    expu[:, st * SW:(st + 1) * SW],
                    logits[b][:, st * SW:(st + 1) * SW],
                    Exp, bias=negmx[:], accum_out=z4[:, st:st + 1],
                )
                for sc in range(4 * st, 4 * (st + 1)):
                    nc.tensor.matmul(
                        psum_t[:, sc:sc + 1],
                        lhsT=expu[:, sc * 128:(sc + 1) * 128],
                        rhs=ones_sb[:],
                        start=True,
                        stop=True,
                    )
            z = sc_pool.tile([1, 1], f32, tag="z")
            nc.vector.reduce_sum(z[:], z4[:], axis=AX)
            rz = sc_pool.tile([1, 1], f32, tag="rz")
            nc.vector.reciprocal(rz[:], z[:])
            # normalized attention weights output (off the critical path)
            attn = soft_pool.tile([1, S], f32, tag="attn")
            nc.vector.tensor_scalar_mul(attn[:], expu[:], rz[:])
            nc.sync.dma_start(out=attn_d[b:b + 1, :], in_=attn[:])
            attnT = soft_pool.tile([128, SC], bf16, tag="attnT")
            nc.vector.tensor_copy(attnT[:], psum_t[:])
            # out[e] = (1/Z) sum_s expu[s] enc[s, e]
            psum_o = [po_pool.tile([1, 512], f32, tag="po", name=f"po{eh_}")
                      for eh_ in range(EH)]
            for sc in range(SC):
                e6 = enc6_pool.tile([128, E], bf16, tag="enc6")
                dmae[(b * SC + sc) % 3].dma_start(
                    out=e6[:],
                    in_=enc[b, sc * 128:(sc + 1) * 128, :],
                )
                for eh in range(EH):
                    nc.tensor.matmul(
                        psum_o[eh][:],
                        lhsT=attnT[:, sc:sc + 1],
                        rhs=e6[:, eh * 512:(eh + 1) * 512],
                        start=(sc == 0),
                        stop=(sc == SC - 1),
                    )
            out_sb = soft_pool.tile([1, E], f32, tag="out_sb")
            for eh in range(EH):
                nc.scalar.activation(
                    out_sb[:, eh * 512:(eh + 1) * 512], psum_o[eh][:],
                    Copy, scale=rz[:])
            nc.sync.dma_start(out=out_d[b:b + 1, :], in_=out_sb[:])

        # pipeline: keep PE busy with batch b+1's matmuls while batch b's
        # softmax chain (DVE/ACT) completes
        step_a(0)
        for b in range(BL):
            if b + 1 < BL:
                step_a(b + 1)
            soft6(b)


def _build():
    import concourse.bacc as bacc
    import concourse.tile as tile
    from concourse import mybir

    f32 = mybir.dt.float32
    bf16 = mybir.dt.bfloat16

    nc = bacc.Bacc("TRN2", target_bir_lowering=False, debug=False,
                   num_devices=N_CORES)
    encT2 = nc.dram_tensor("encT2", [BL, 128, EC, ST, SW], bf16,
                           kind="ExternalInput").ap()
    enc = nc.dram_tensor("enc", [BL, S, E], bf16, kind="ExternalInput").ap()
    wencT = nc.dram_tensor("wencT", [E, H], bf16, kind="ExternalInput").ap()
    dprojT = nc.dram_tensor("dprojT", [128, HT * BL], f32,
                            kind="ExternalInput").ap()
    vTf = nc.dram_tensor("vTf", [128, HT], f32, kind="ExternalInput").ap()
    maskb = nc.dram_tensor("maskb", [BL, S], f32, kind="ExternalInput").ap()
    out_d = nc.dram_tensor("out", [BL, E], f32, kind="ExternalOutput").ap()
    attn_d = nc.dram_tensor("attnw", [BL, S], f32, kind="ExternalOutput").ap()

    with tile.TileContext(nc) as tc:
        _emit(tc, mybir, encT2, enc, wencT, dprojT, vTf, maskb, out_d, attn_d)
    nc.compile()
    return nc


def get_compiled():
    if "nc" not in _CACHE:
        _CACHE["nc"] = _build()
    return _CACHE["nc"]


def make_in_maps(encoder_outputs, sequence_length, decoder_state, W, v):
    encoder_outputs = np.asarray(encoder_outputs, dtype=np.float32)
    sequence_length = np.asarray(sequence_length)
    decoder_state = np.asarray(decoder_state, dtype=np.float32)
    W = np.asarray(W, dtype=np.float32)
    v = np.asarray(v, dtype=np.float32)

    wencT = np.ascontiguousarray(W[:, :E].T).astype(BF16)    # [e, h]
    vTf = np.ascontiguousarray(v.reshape(HT, 128).T).astype(np.float32)
    mask = np.where(
        np.arange(S)[None, :] < sequence_length[:, None].astype(np.int64),
        np.float32(0.0), np.float32(MASK_VAL),
    ).astype(np.float32)                                     # [B, S]
    # dec_proj[b, h] = decoder_state @ W_dec.T   (0.05% of the FLOPs)
    dproj = decoder_state @ W[:, E:].T                       # [B, H] f32
    # device layout: [h%128, (h//128)*BL + b]
    dproj_t = dproj.T.reshape(HT, 128, B)                    # [t, p, b]

    in_maps = []
    for cid in range(N_CORES):
        sl = slice(cid * BL, (cid + 1) * BL)
        enc_c = encoder_outputs[sl]
        encT2 = (enc_c.transpose(0, 2, 1)            # [BL, E, S]
                 .reshape(BL, EC, 128, ST, SW)       # e=(c,p), s=(st,s')
                 .transpose(0, 2, 1, 3, 4)           # [BL, 128, EC, ST, SW]
                 .astype(BF16))
        dpt = np.ascontiguousarray(
            dproj_t[:, :, sl].transpose(1, 0, 2).reshape(128, HT * BL)
        ).astype(np.float32)
        in_maps.append({
            "encT2": np.ascontiguousarray(encT2),
            "enc": enc_c.astype(BF16),
            "wencT": wencT,
            "dprojT": dpt,
            "vTf": vTf,
            "maskb": np.ascontiguousarray(mask[sl]),
        })
    return in_maps


def run(in_maps, trace=False, **kw):
    from concourse.bass_utils import run_bass_kernel_spmd

    nc = get_compiled()
    return run_bass_kernel_spmd(nc, in_maps, list(range(N_CORES)), trace=trace, **kw)


def kernel(encoder_outputs, sequence_length, decoder_state, W, v):
    in_maps = make_in_maps(encoder_outputs, sequence_length, decoder_state, W, v)
    res = run(in_maps)
    out = np.concatenate(
        [np.asarray(res.results[i]["out"], dtype=np.float32) for i in range(N_CORES)], axis=0)
    attn = np.concatenate(
        [np.asarray(res.results[i]["attnw"], dtype=np.float32) for i in range(N_CORES)], axis=0)
    return out, attn


# revision 7
# speedup vs baseline: 1.8436x; 1.5891x over previous
"""ConcatAttention (Bahdanau-style) Trainium2 kernel.

score = v^T tanh(W [h; s]);  softmax over masked sequence;  out = attn @ h.

Slot-sharded: the (batch, s-tile-of-512) pairs that contain any valid
(unmasked) position form "slots"; masked tiles are skipped entirely (their
attention weights are exactly 0 because exp(-50000 - x) underflows).  Slots
are dealt round-robin across the 8 NeuronCores for near-perfect balance.

Per slot: enc_proj matmul in bf16 ([512, 1024] x [1024, 1024]) with the
decoder projection fused as the tanh bias; v-contraction on the Vector
engine; exp WITHOUT max-subtraction (|logit| <= ||v||_1 ~ 25, so fp32 exp
cannot overflow), which makes every slot independent; per-slot partial
outputs o_i = sum_s exp(l_s) enc[s] via attn-stationary bf16 matmuls.
The tiny cross-slot combine (out[b] = sum o_i / sum z_i, attn = expu/Z)
is the host-side unshard step.
"""

import numpy as np
import ml_dtypes

BF16 = ml_dtypes.bfloat16

B, S, E, H = 32, 2048, 1024, 1024
D = 1024  # decoder dim (IN_FEATURES - E)
N_CORES = 8
MASK_VAL = -50000.0

SW = 512      # slot width (s positions per slot)
NST = S // SW  # 4 s-tiles per batch max
HT = 8        # h tiles of 128
EC = 8        # e (contraction) chunks of 128
NSC = SW // 128  # 4 s-chunks of 128 per slot (step B)
EH = 2        # e halves of 512 (step B)

_CACHE = {}


def _emit(tc, mybir, nt, encT3, enc6s, wencT3, dprojT, vTf, masks,
          o_d, expu_d, z_d):
    nc = tc.nc
    f32 = mybir.dt.float32
    bf16 = mybir.dt.bfloat16
    Tanh = mybir.ActivationFunctionType.Tanh
    Exp = mybir.ActivationFunctionType.Exp
    Copy = mybir.ActivationFunctionType.Copy
    mult = mybir.AluOpType.mult
    add = mybir.AluOpType.add

    dmae = [nc.sync, nc.scalar, nc.gpsimd]

    from contextlib import ExitStack

    with ExitStack() as ctx:
        consts = ctx.enter_context(tc.tile_pool(name="consts", bufs=1))
        encT_pool = ctx.enter_context(tc.tile_pool(name="encT", bufs=3))
        enc6_pool = ctx.enter_context(tc.tile_pool(name="enc6", bufs=3))
        hid_pool = ctx.enter_context(tc.tile_pool(name="hid", bufs=3))
        acc_pool = ctx.enter_context(tc.tile_pool(name="acc", bufs=2))
        soft_pool = ctx.enter_context(tc.tile_pool(name="soft", bufs=3))
        pa_pool = ctx.enter_context(tc.tile_pool(name="pa", bufs=2, space="PSUM"))
        pl_pool = ctx.enter_context(tc.tile_pool(name="pl", bufs=2, space="PSUM"))
        pt_pool = ctx.enter_context(tc.tile_pool(name="pt", bufs=2, space="PSUM"))
        po_pool = ctx.enter_context(tc.tile_pool(name="po", bufs=2, space="PSUM"))

        # step-A weights: [p, c*H + h], 8 KiB/partition halves on two queues
        wenc_sb = consts.tile([128, EC * H], bf16)
        for half in range(2):
            dmae[half].dma_start(
                out=wenc_sb[:, half * 4 * H:(half + 1) * 4 * H],
                in_=wencT3[:, half * 4:(half + 1) * 4, :],
            )
        vf_sb = consts.tile([128, HT], f32)
        nc.gpsimd.dma_start(out=vf_sb[:], in_=vTf[:, :])
        # host-computed dec projection, per-slot columns: [p, t*nt + i]
        dproj = consts.tile([128, HT * nt], f32)
        nc.gpsimd.dma_start(out=dproj[:], in_=dprojT[:, :])
        ones_sb = consts.tile([1, 1], f32)
        nc.vector.memset(ones_sb[:], 1.0)
        ones_bf = consts.tile([128, 1], bf16)
        nc.vector.memset(ones_bf[:], 1.0)
        z_sb = consts.tile([1, nt], f32)

        for i in range(nt):
            # ---- step A: logits for this slot ----
            et = encT_pool.tile([128, EC * SW], bf16, tag="encT")
            for half in range(2):
                dmae[(2 * i + half) % 3].dma_start(
                    out=et[:, half * 4 * SW:(half + 1) * 4 * SW],
                    in_=encT3[i, :, half * 4:(half + 1) * 4, :],
                )
            mask_i = soft_pool.tile([1, SW], f32, tag="mask")
            nc.gpsimd.dma_start(out=mask_i[:], in_=masks[i:i + 1, :])
            acc = acc_pool.tile([128, SW], f32, tag="acc")
            acc_bf = acc_pool.tile([128, SW], bf16, tag="accbf")
            for t in range(HT):
                psum_a = pa_pool.tile([128, SW], f32, tag="pa")
                for c in range(EC):
                    nc.tensor.matmul(
                        psum_a[:],
                        lhsT=wenc_sb[:, c * H + t * 128: c * H + (t + 1) * 128],
                        rhs=et[:, c * SW:(c + 1) * SW],
                        start=(c == 0),
                        stop=(c == EC - 1),
                    )
                hid = hid_pool.tile([128, SW], bf16, tag="hid")
                nc.scalar.activation(
                    hid[:], psum_a[:], Tanh,
                    bias=dproj[:, t * nt + i: t * nt + i + 1],
                )
                if t == 0:
                    nc.vector.tensor_scalar_mul(acc[:], hid[:], vf_sb[:, 0:1])
                elif t < HT - 1:
                    nc.vector.scalar_tensor_tensor(
                        acc[:], hid[:], vf_sb[:, t:t + 1], acc[:], mult, add)
                else:
                    nc.vector.scalar_tensor_tensor(
                        acc_bf[:], hid[:], vf_sb[:, t:t + 1], acc[:], mult, add)
            psum_l = pl_pool.tile([1, SW], f32, tag="pl")
            nc.tensor.matmul(psum_l[:], lhsT=ones_bf[:], rhs=acc_bf[:],
                             start=True, stop=True)
            lg = soft_pool.tile([1, SW], f32, tag="lg")
            nc.vector.tensor_add(lg[:], psum_l[:], mask_i[:])
            # ---- exp without max-subtraction; z_i via the ACT accumulator ----
            expu = soft_pool.tile([1, SW], f32, tag="expu")
            nc.scalar.activation(expu[:], lg[:], Exp,
                                 accum_out=z_sb[:, i:i + 1])
            nc.sync.dma_start(out=expu_d[i:i + 1, :], in_=expu[:])
            # ---- transpose expu to [s%128, s//128] via k=1 matmuls ----
            psum_t = pt_pool.tile([128, NSC], f32, tag="pt")
            for sc in range(NSC):
                nc.tensor.matmul(
                    psum_t[:, sc:sc + 1],
                    lhsT=expu[:, sc * 128:(sc + 1) * 128],
                    rhs=ones_sb[:],
                    start=True,
                    stop=True,
                )
            attnT = soft_pool.tile([128, NSC], bf16, tag="attnT")
            nc.vector.tensor_copy(attnT[:], psum_t[:])
            # ---- o_i[e] = sum_s expu[s] enc[s, e] ----
            e6 = enc6_pool.tile([128, NSC * E], bf16, tag="enc6")
            for half in range(2):
                dmae[(2 * i + 1 + half) % 3].dma_start(
                    out=e6[:, half * 2 * E:(half + 1) * 2 * E],
                    in_=enc6s[i, :, half * 2:(half + 1) * 2, :],
                )
            psum_o = [po_pool.tile([1, 512], f32, tag="po", name=f"po{eh_}")
                      for eh_ in range(EH)]
            for sc in range(NSC):
                for eh in range(EH):
                    nc.tensor.matmul(
                        psum_o[eh][:],
                        lhsT=attnT[:, sc:sc + 1],
                        rhs=e6[:, sc * E + eh * 512: sc * E + (eh + 1) * 512],
                        start=(sc == 0),
                        stop=(sc == NSC - 1),
                    )
            osb = soft_pool.tile([1, E], f32, tag="osb")
            for eh in range(EH):
                nc.scalar.activation(osb[:, eh * 512:(eh + 1) * 512],
                                     psum_o[eh][:], Copy)
            nc.sync.dma_start(out=o_d[i:i + 1, :], in_=osb[:])

        nc.sync.dma_start(out=z_d[:, :], in_=z_sb[:])


def _build(nt):
    import concourse.bacc as bacc
    import concourse.tile as tile
    from concourse import mybir

    f32 = mybir.dt.float32
    bf16 = mybir.dt.bfloat16

    nc = bacc.Bacc("TRN2", target_bir_lowering=False, debug=False,
                   num_devices=N_CORES)
    encT3 = nc.dram_tensor("encT3", [nt, 128, EC, SW], bf16,
                           kind="ExternalInput").ap()
    enc6s = nc.dram_tensor("enc6s", [nt, 128, NSC, E], bf16,
                           kind="ExternalInput").ap()
    wencT3 = nc.dram_tensor("wencT3", [128, EC, H], bf16,
                            kind="ExternalInput").ap()
    dprojT = nc.dram_tensor("dprojT", [128, HT * nt], f32,
                            kind="ExternalInput").ap()
    vTf = nc.dram_tensor("vTf", [128, HT], f32, kind="ExternalInput").ap()
    masks = nc.dram_tensor("masks", [nt, SW], f32, kind="ExternalInput").ap()
    o_d = nc.dram_tensor("o_slots", [nt, E], f32, kind="ExternalOutput").ap()
    expu_d = nc.dram_tensor("expu_slots", [nt, SW], f32,
                            kind="ExternalOutput").ap()
    z_d = nc.dram_tensor("z_slots", [1, nt], f32, kind="ExternalOutput").ap()

    with tile.TileContext(nc) as tc:
        _emit(tc, mybir, nt, encT3, enc6s, wencT3, dprojT, vTf, masks,
              o_d, expu_d, z_d)
    nc.compile()
    return nc


def get_compiled(nt):
    if nt not in _CACHE:
        _CACHE[nt] = _build(nt)
    return _CACHE[nt]


def plan_slots(sequence_length):
    """All (batch, s-tile) pairs with any valid position, dealt round-robin."""
    slots = []
    for b in range(B):
        for st in range(int(np.ceil(sequence_length[b] / SW))):
            slots.append((b, st))
    nt = max(1, (len(slots) + N_CORES - 1) // N_CORES)
    per_core = [slots[c::N_CORES] for c in range(N_CORES)]
    return per_core, nt


def make_in_maps(encoder_outputs, sequence_length, decoder_state, W, v):
    encoder_outputs = np.asarray(encoder_outputs, dtype=np.float32)
    sequence_length = np.asarray(sequence_length).astype(np.int64)
    decoder_state = np.asarray(decoder_state, dtype=np.float32)
    W = np.asarray(W, dtype=np.float32)
    v = np.asarray(v, dtype=np.float32)

    per_core, nt = plan_slots(sequence_length)

    wencT3 = np.ascontiguousarray(
        W[:, :E].T.reshape(EC, 128, H).transpose(1, 0, 2)).astype(BF16)
    vTf = np.ascontiguousarray(v.reshape(HT, 128).T).astype(np.float32)
    dproj = decoder_state @ W[:, E:].T                   # [B, H] f32
    enc_bf = encoder_outputs.astype(BF16)                # [B, S, E]
    # transposed copy for step A, tiled: [B, ST, 128(p), EC, SW]
    encT_all = np.ascontiguousarray(
        enc_bf.transpose(0, 2, 1)                        # [B, E, S]
        .reshape(B, EC, 128, NST, SW)
        .transpose(0, 3, 2, 1, 4))                       # [B, ST, 128, EC, SW]
    svalid = np.arange(S).reshape(NST, SW)

    in_maps = []
    for cid in range(N_CORES):
        slots = per_core[cid]
        encT3 = np.zeros((nt, 128, EC, SW), BF16)
        enc6s = np.zeros((nt, 128, NSC, E), BF16)
        masks = np.full((nt, SW), MASK_VAL, np.float32)
        dpt = np.zeros((128, HT, nt), np.float32)
        for i, (b, st) in enumerate(slots):
            encT3[i] = encT_all[b, st]
            enc6s[i] = enc_bf[b, st * SW:(st + 1) * SW, :].reshape(
                NSC, 128, E).transpose(1, 0, 2)
            masks[i] = np.where(svalid[st] < sequence_length[b], 0.0, MASK_VAL)
            dpt[:, :, i] = dproj[b].reshape(HT, 128).T
        in_maps.append({
            "encT3": encT3,
            "enc6s": enc6s,
            "wencT3": wencT3,
            "dprojT": np.ascontiguousarray(dpt.reshape(128, HT * nt)),
            "vTf": vTf,
            "masks": masks,
        })
    return in_maps, per_core, nt


def combine(results, per_core, sequence_length):
    """Host-side unshard: out[b] = sum_i o_i / Z_b, attn = expu / Z_b."""
    Z = np.zeros(B, np.float64)
    out = np.zeros((B, E), np.float64)
    attn = np.zeros((B, S), np.float32)
    for cid in range(N_CORES):
        r = results[cid]
        o = np.asarray(r["o_slots"], np.float64)
        ex = np.asarray(r["expu_slots"], np.float32)
        z = np.asarray(r["z_slots"], np.float64).reshape(-1)
        for i, (b, st) in enumerate(per_core[cid]):
            Z[b] += z[i]
            out[b] += o[i]
            attn[b, st * SW:(st + 1) * SW] = ex[i]
    out = (out / Z[:, None]).astype(np.float32)
    attn = attn / Z[:, None].astype(np.float32)
    return out, attn.astype(np.float32)


def run(in_maps, nt, trace=False, **kw):
    from concourse.bass_utils import run_bass_kernel_spmd

    nc = get_compiled(nt)
    return run_bass_kernel_spmd(nc, in_maps, list(range(N_CORES)), trace=trace, **kw)


def kernel(encoder_outputs, sequence_length, decoder_state, W, v):
    in_maps, per_core, nt = make_in_maps(
        encoder_outputs, sequence_length, decoder_state, W, v)
    res = run(in_maps, nt)
    return combine(res.results, per_core, np.asarray(sequence_length))


# revision 8
# speedup vs baseline: 1.8683x; 1.0134x over previous
"""ConcatAttention (Bahdanau-style) Trainium2 kernel.

score = v^T tanh(W [h; s]);  softmax over masked sequence;  out = attn @ h.

Slot-sharded: the (batch, s-tile-of-512) pairs that contain any valid
(unmasked) position form "slots"; fully-masked tiles are skipped entirely
(their attention weights are exactly 0).  Slots are dealt round-robin across
the 8 NeuronCores for near-perfect balance; the tiny cross-slot softmax
combine (flash-attention style, using per-slot max m_i and sum z_i) is the
host-side unshard step.

Per slot, on device: enc_proj matmul in fp16 ([512,1024] x [1024,1024]) with
the decoder projection fused as the tanh bias; v-contraction on the Vector
engine in fp16 (2x mode) finished by a ones-vector matmul; slot-local
softmax numerator exp(l - m_i) in fp16; the rank-1 weighted sum
o_i[e] = sum_s w_s encT[e, s] runs on the Vector engine against the
already-resident transposed tile (no second encoder copy), with the weights
broadcast across partitions via a DRAM bounce.
"""

import numpy as np
import ml_dtypes

BF16 = ml_dtypes.bfloat16
F16 = np.float16

B, S, E, H = 32, 2048, 1024, 1024
D = 1024  # decoder dim (IN_FEATURES - E)
N_CORES = 8
MASK_VAL = -50000.0

SW = 512      # slot width (s positions per slot)
NST = S // SW  # 4 s-tiles per batch max
HT = 8        # h tiles of 128
EC = 8        # e (contraction) chunks of 128

_CACHE = {}


def _emit(tc, mybir, nt, encT3, wencT3, dprojT, vTf, masks,
          o_d, expu_d, m_d, z_d):
    nc = tc.nc
    f32 = mybir.dt.float32
    f16 = mybir.dt.float16
    Tanh = mybir.ActivationFunctionType.Tanh
    Exp = mybir.ActivationFunctionType.Exp
    AX = mybir.AxisListType.X
    mult = mybir.AluOpType.mult
    add = mybir.AluOpType.add

    import concourse.bass as bass
    dmae = [nc.sync, nc.scalar, nc.gpsimd]

    from contextlib import ExitStack

    with ExitStack() as ctx:
        consts = ctx.enter_context(tc.tile_pool(name="consts", bufs=1))
        encT_pool = ctx.enter_context(tc.tile_pool(name="encT", bufs=4))
        hid_pool = ctx.enter_context(tc.tile_pool(name="hid", bufs=3))
        acc_pool = ctx.enter_context(tc.tile_pool(name="acc", bufs=2))
        soft_pool = ctx.enter_context(tc.tile_pool(name="soft", bufs=3))
        wb_pool = ctx.enter_context(tc.tile_pool(name="wb", bufs=2))
        dram_pool = ctx.enter_context(tc.tile_pool(name="dram", bufs=2,
                                                   space="DRAM"))
        pa_pool = ctx.enter_context(tc.tile_pool(name="pa", bufs=3, space="PSUM"))
        pl_pool = ctx.enter_context(tc.tile_pool(name="pl", bufs=2, space="PSUM"))

        # step-A weights, t-major ([p, t*EC*128 + c*128 + h']), t=0 first so
        # the first t-group's weights land with the first enc tile
        wenc_sb = consts.tile([128, HT * EC * 128], f16)
        for t in range(HT):
            dmae[(t + 1) % 3].dma_start(
                out=wenc_sb[:, t * 1024:(t + 1) * 1024],
                in_=wencT3[:, t, :, :],
            )
        vf_sb = consts.tile([128, HT], f32)
        nc.gpsimd.dma_start(out=vf_sb[:], in_=vTf[:, :])
        dproj = consts.tile([128, HT * nt], f32)
        nc.gpsimd.dma_start(out=dproj[:], in_=dprojT[:, :])
        ones16 = consts.tile([128, 1], f16)
        nc.vector.memset(ones16[:], 1.0)
        z_sb = consts.tile([1, nt], f32)
        m_sb = consts.tile([1, nt], f32)

        for i in range(nt):
            # ---- step A: logits for this slot ----
            et = encT_pool.tile([128, EC * SW], f16, tag="encT")
            for q in range(4):
                dmae[q % 3].dma_start(
                    out=et[:, q * 2 * SW:(q + 1) * 2 * SW],
                    in_=encT3[i, :, q * 2:(q + 1) * 2, :],
                )
            mask_i = soft_pool.tile([1, SW], f32, tag="mask")
            nc.gpsimd.dma_start(out=mask_i[:], in_=masks[i:i + 1, :])
            acc = acc_pool.tile([128, SW], f16, tag="acc")
            for t in range(HT):
                psum_a = pa_pool.tile([128, SW], f32, tag="pa")
                for c in range(EC):
                    nc.tensor.matmul(
                        psum_a[:],
                        lhsT=wenc_sb[:, t * 1024 + c * 128: t * 1024 + (c + 1) * 128],
                        rhs=et[:, c * SW:(c + 1) * SW],
                        start=(c == 0),
                        stop=(c == EC - 1),
                    )
                hid = hid_pool.tile([128, SW], f16, tag="hid")
                nc.scalar.activation(
                    hid[:], psum_a[:], Tanh,
                    bias=dproj[:, t * nt + i: t * nt + i + 1],
                )
                if t == 0:
                    nc.vector.tensor_scalar_mul(acc[:], hid[:], vf_sb[:, 0:1])
                else:
                    nc.vector.scalar_tensor_tensor(
                        acc[:], hid[:], vf_sb[:, t:t + 1], acc[:], mult, add)
            psum_l = pl_pool.tile([1, SW], f32, tag="pl")
            nc.tensor.matmul(psum_l[:], lhsT=ones16[:], rhs=acc[:],
                             start=True, stop=True)
            lg = soft_pool.tile([1, SW], f32, tag="lg")
            nc.vector.tensor_add(lg[:], psum_l[:], mask_i[:])
            # ---- slot-local max and exp(l - m_i); z_i via ACT accumulator ----
            nc.vector.reduce_max(m_sb[:, i:i + 1], lg[:], axis=AX)
            negm = soft_pool.tile([1, 1], f32, tag="negm")
            nc.vector.tensor_scalar_mul(negm[:], m_sb[:, i:i + 1], -1.0)
            expu = soft_pool.tile([1, SW], f16, tag="expu")
            nc.scalar.activation(expu[:], lg[:], Exp, bias=negm[:],
                                 accum_out=z_sb[:, i:i + 1])
            nc.sync.dma_start(out=expu_d[i:i + 1, :], in_=expu[:])
            # ---- broadcast weights across partitions via DRAM bounce ----
            ebounce = dram_pool.tile([1, SW], f16, tag="ebounce")
            nc.scalar.dma_start(out=ebounce[:], in_=expu[:])
            w_bc = wb_pool.tile([128, SW], f16, tag="wbc")
            src = bass.AP(
                tensor=ebounce.tensor,
                offset=ebounce.offset,
                ap=[[0, 128]] + list(ebounce.ap),
            )
            nc.gpsimd.dma_start(out=w_bc[:], in_=src)
            # ---- o_i[e] = sum_s w_s encT[e, s] on the Vector engine ----
            ot = soft_pool.tile([128, EC], f32, tag="ot")
            for c in range(EC):
                scr = wb_pool.tile([128, SW], f16, tag="scr")
                nc.vector.scalar_tensor_tensor(
                    scr[:], et[:, c * SW:(c + 1) * SW], 1.0, w_bc[:],
                    mult, mult, accum_out=ot[:, c:c + 1])
            nc.sync.dma_start(out=o_d[i, :, :], in_=ot[:])

        nc.sync.dma_start(out=z_d[:, :], in_=z_sb[:])
        nc.sync.dma_start(out=m_d[:, :], in_=m_sb[:])


def _build(nt):
    import concourse.bacc as bacc
    import concourse.tile as tile
    from concourse import mybir

    f32 = mybir.dt.float32
    f16 = mybir.dt.float16

    nc = bacc.Bacc("TRN2", target_bir_lowering=False, debug=False,
                   num_devices=N_CORES)
    encT3 = nc.dram_tensor("encT3", [nt, 128, EC, SW], f16,
                           kind="ExternalInput").ap()
    wencT3 = nc.dram_tensor("wencT3", [128, HT, EC, 128], f16,
                            kind="ExternalInput").ap()
    dprojT = nc.dram_tensor("dprojT", [128, HT * nt], f32,
                            kind="ExternalInput").ap()
    vTf = nc.dram_tensor("vTf", [128, HT], f32, kind="ExternalInput").ap()
    masks = nc.dram_tensor("masks", [nt, SW], f32, kind="ExternalInput").ap()
    o_d = nc.dram_tensor("o_slots", [nt, 128, EC], f32,
                         kind="ExternalOutput").ap()
    expu_d = nc.dram_tensor("expu_slots", [nt, SW], f16,
                            kind="ExternalOutput").ap()
    m_d = nc.dram_tensor("m_slots", [1, nt], f32, kind="ExternalOutput").ap()
    z_d = nc.dram_tensor("z_slots", [1, nt], f32, kind="ExternalOutput").ap()

    with tile.TileContext(nc) as tc:
        _emit(tc, mybir, nt, encT3, wencT3, dprojT, vTf, masks,
              o_d, expu_d, m_d, z_d)
    nc.compile()
    return nc


def get_compiled(nt):
    if nt not in _CACHE:
        _CACHE[nt] = _build(nt)
    return _CACHE[nt]


def plan_slots(sequence_length):
    """All (batch, s-tile) pairs with any valid position, dealt round-robin."""
    slots = []
    for b in range(B):
        for st in range(int(np.ceil(sequence_length[b] / SW))):
            slots.append((b, st))
    nt = max(1, (len(slots) + N_CORES - 1) // N_CORES)
    per_core = [slots[c::N_CORES] for c in range(N_CORES)]
    return per_core, nt


def make_in_maps(encoder_outputs, sequence_length, decoder_state, W, v):
    encoder_outputs = np.asarray(encoder_outputs, dtype=np.float32)
    sequence_length = np.asarray(sequence_length).astype(np.int64)
    decoder_state = np.asarray(decoder_state, dtype=np.float32)
    W = np.asarray(W, dtype=np.float32)
    v = np.asarray(v, dtype=np.float32)

    per_core, nt = plan_slots(sequence_length)

    # [p, t(h//128), c(e//128), h%128]
    wencT3 = np.ascontiguousarray(
        W[:, :E].reshape(HT, 128, EC, 128).transpose(3, 0, 2, 1)).astype(F16)
    vTf = np.ascontiguousarray(v.reshape(HT, 128).T).astype(np.float32)
    dproj = decoder_state @ W[:, E:].T                   # [B, H] f32
    # transposed copy for step A, tiled: [B, ST, 128(p), EC, SW]
    encT_all = np.ascontiguousarray(
        encoder_outputs.astype(F16).transpose(0, 2, 1)   # [B, E, S]
        .reshape(B, EC, 128, NST, SW)
        .transpose(0, 3, 2, 1, 4))                       # [B, ST, 128, EC, SW]
    svalid = np.arange(S).reshape(NST, SW)

    in_maps = []
    for cid in range(N_CORES):
        slots = per_core[cid]
        encT3 = np.zeros((nt, 128, EC, SW), F16)
        masks = np.full((nt, SW), MASK_VAL, np.float32)
        dpt = np.zeros((128, HT, nt), np.float32)
        for i, (b, st) in enumerate(slots):
            encT3[i] = encT_all[b, st]
            masks[i] = np.where(svalid[st] < sequence_length[b], 0.0, MASK_VAL)
            dpt[:, :, i] = dproj[b].reshape(HT, 128).T
        in_maps.append({
            "encT3": encT3,
            "wencT3": wencT3,
            "dprojT": np.ascontiguousarray(
                dpt.transpose(0, 1, 2).reshape(128, HT * nt)),
            "vTf": vTf,
            "masks": masks,
        })
    return in_maps, per_core, nt


def combine(results, per_core, sequence_length):
    """Host-side unshard: flash-style softmax combine across slots."""
    M = np.full(B, -np.inf)
    for cid in range(N_CORES):
        m = np.asarray(results[cid]["m_slots"], np.float64).reshape(-1)
        for i, (b, st) in enumerate(per_core[cid]):
            M[b] = max(M[b], m[i])
    Z = np.zeros(B, np.float64)
    out = np.zeros((B, E), np.float64)
    attn = np.zeros((B, S), np.float32)
    for cid in range(N_CORES):
        r = results[cid]
        o = np.asarray(r["o_slots"], np.float64)         # [nt, 128, EC]
        ex = np.asarray(r["expu_slots"], np.float32)     # [nt, SW] f16->f32
        z = np.asarray(r["z_slots"], np.float64).reshape(-1)
        m = np.asarray(r["m_slots"], np.float64).reshape(-1)
        for i, (b, st) in enumerate(per_core[cid]):
            w = np.exp(m[i] - M[b])
            Z[b] += w * z[i]
            out[b] += w * o[i].T.reshape(E)              # e = c*128 + p
            attn[b, st * SW:(st + 1) * SW] = ex[i] * np.float32(w)
    out = (out / Z[:, None]).astype(np.float32)
    attn = attn / Z[:, None].astype(np.float32)
    return out, attn.astype(np.float32)


def run(in_maps, nt, trace=False, **kw):
    from concourse.bass_utils import run_bass_kernel_spmd

    nc = get_compiled(nt)
    return run_bass_kernel_spmd(nc, in_maps, list(range(N_CORES)), trace=trace, **kw)


def kernel(encoder_outputs, sequence_length, decoder_state, W, v):
    in_maps, per_core, nt = make_in_maps(
        encoder_outputs, sequence_length, decoder_state, W, v)
    res = run(in_maps, nt)
    return combine(res.results, per_core, np.asarray(sequence_length))


# revision 13
# speedup vs baseline: 2.0629x; 1.1042x over previous
"""ConcatAttention (Bahdanau-style) Trainium2 kernel.

score = v^T tanh(W [h; s]);  softmax over masked sequence;  out = attn @ h.

Slot-sharded: the (batch, s-tile-of-512) pairs that contain any valid
(unmasked) position form "slots"; fully-masked tiles are skipped entirely
(their attention weights are exactly 0).  Slots are dealt round-robin across
the 8 NeuronCores for near-perfect balance; the tiny cross-slot softmax
combine (flash-attention style, using per-slot max m_i and sum z_i) is the
host-side unshard step.

Per slot, on device: enc_proj matmul in fp16 ([512,1024] x [1024,1024]) with
the decoder projection fused as the tanh bias; v-contraction on the Vector
engine in fp16 (2x mode) finished by a ones-vector matmul; slot-local
softmax numerator exp(l - m_i) in fp16; the rank-1 weighted sum
o_i[e] = sum_s w_s encT[e, s] runs on the Vector engine against the
already-resident transposed tile (no second encoder copy), with the weights
broadcast across partitions via a DRAM bounce.
"""

import numpy as np
import ml_dtypes

BF16 = ml_dtypes.bfloat16
F16 = np.float16

B, S, E, H = 32, 2048, 1024, 1024
D = 1024  # decoder dim (IN_FEATURES - E)
N_CORES = 8
MASK_VAL = -50000.0

SW = 512      # slot width (s positions per slot)
NST = S // SW  # 4 s-tiles per batch max
HT = 8        # h tiles of 128
EC = 8        # e (contraction) chunks of 128

_CACHE = {}


def _emit(tc, mybir, nt, encT3, wencT3, dprojT, vTf, masks,
          o_d, expu_d, m_d, z_d):
    nc = tc.nc
    f32 = mybir.dt.float32
    f16 = mybir.dt.float16
    Tanh = mybir.ActivationFunctionType.Tanh
    Exp = mybir.ActivationFunctionType.Exp
    Copy = mybir.ActivationFunctionType.Copy
    AX = mybir.AxisListType.X
    mult = mybir.AluOpType.mult
    add = mybir.AluOpType.add

    import concourse.bass as bass
    dmae = [nc.sync, nc.scalar, nc.gpsimd]

    from contextlib import ExitStack

    with ExitStack() as ctx:
        consts = ctx.enter_context(tc.tile_pool(name="consts", bufs=1))
        encT_pool = ctx.enter_context(tc.tile_pool(name="encT", bufs=5))
        hid_pool = ctx.enter_context(tc.tile_pool(name="hid", bufs=3))
        acc_pool = ctx.enter_context(tc.tile_pool(name="acc", bufs=2))
        soft_pool = ctx.enter_context(tc.tile_pool(name="soft", bufs=3))
        wb_pool = ctx.enter_context(tc.tile_pool(name="wb", bufs=2))
        pa_pool = ctx.enter_context(tc.tile_pool(name="pa", bufs=3, space="PSUM"))
        pl_pool = ctx.enter_context(tc.tile_pool(name="pl", bufs=2, space="PSUM"))
        pb_pool = ctx.enter_context(tc.tile_pool(name="pb", bufs=2, space="PSUM"))

        # step-A weights, t-major ([p, t*EC*128 + c*128 + h']), t=0 first so
        # the first t-group's weights land with the first enc tile
        wenc_sb = consts.tile([128, HT * EC * 128], f16)
        for t in range(HT):
            dmae[(t + 1) % 3].dma_start(
                out=wenc_sb[:, t * 1024:(t + 1) * 1024],
                in_=wencT3[:, t, :, :],
            )
        vf_sb = consts.tile([128, HT], f32)
        nc.gpsimd.dma_start(out=vf_sb[:], in_=vTf[:, :])
        dproj = consts.tile([128, HT * nt], f32)
        nc.gpsimd.dma_start(out=dproj[:], in_=dprojT[:, :])
        ones16 = consts.tile([128, 1], f16)
        nc.vector.memset(ones16[:], 1.0)
        onesrow = consts.tile([1, 128], f16)
        nc.vector.memset(onesrow[:], 1.0)
        z_sb = consts.tile([1, nt], f32)
        m_sb = consts.tile([1, nt], f32)
        mask_all = consts.tile([1, nt * SW], f32)
        nc.gpsimd.dma_start(out=mask_all[:], in_=masks[:, :])

        for i in range(nt):
            # ---- step A: logits for this slot ----
            et = encT_pool.tile([128, EC * SW], f16, tag="encT")
            for q in range(4):
                dmae[q % 3].dma_start(
                    out=et[:, q * 2 * SW:(q + 1) * 2 * SW],
                    in_=encT3[i, :, q * 2:(q + 1) * 2, :],
                )
            acc = acc_pool.tile([128, SW], f16, tag="acc")
            for t in range(HT):
                psum_a = pa_pool.tile([128, SW], f32, tag="pa")
                for c in range(EC):
                    nc.tensor.matmul(
                        psum_a[:],
                        lhsT=wenc_sb[:, t * 1024 + c * 128: t * 1024 + (c + 1) * 128],
                        rhs=et[:, c * SW:(c + 1) * SW],
                        start=(c == 0),
                        stop=(c == EC - 1),
                    )
                hid = hid_pool.tile([128, SW], f16, tag="hid")
                nc.scalar.activation(
                    hid[:], psum_a[:], Tanh,
                    bias=dproj[:, t * nt + i: t * nt + i + 1],
                )
                # v_t * tanh(...) as an ACT scaled copy; accumulate at DVE 2x
                if t == 0:
                    nc.scalar.activation(acc[:], hid[:], Copy,
                                         scale=vf_sb[:, 0:1])
                else:
                    sc = hid_pool.tile([128, SW], f16, tag="sc")
                    nc.scalar.activation(sc[:], hid[:], Copy,
                                         scale=vf_sb[:, t:t + 1])
                    nc.vector.tensor_add(acc[:], acc[:], sc[:])
            psum_l = pl_pool.tile([1, SW], f32, tag="pl")
            nc.tensor.matmul(psum_l[:], lhsT=ones16[:], rhs=acc[:],
                             start=True, stop=True)
            lg = soft_pool.tile([1, SW], f32, tag="lg")
            nc.vector.tensor_add(lg[:], psum_l[:],
                                 mask_all[:, i * SW:(i + 1) * SW])
            # ---- slot-local max and exp(l - m_i); z_i via ACT accumulator ----
            nc.vector.reduce_max(m_sb[:, i:i + 1], lg[:], axis=AX)
            negm = soft_pool.tile([1, 1], f32, tag="negm")
            nc.vector.tensor_scalar_mul(negm[:], m_sb[:, i:i + 1], -1.0)
            expu = soft_pool.tile([1, SW], f16, tag="expu")
            nc.scalar.activation(expu[:], lg[:], Exp, bias=negm[:],
                                 accum_out=z_sb[:, i:i + 1])
            nc.sync.dma_start(out=expu_d[i:i + 1, :], in_=expu[:])
            # ---- broadcast weights across partitions via a k=1 matmul ----
            psum_bc = pb_pool.tile([128, SW], f32, tag="pb")
            nc.tensor.matmul(psum_bc[:], lhsT=onesrow[:], rhs=expu[:],
                             start=True, stop=True)
            # ---- o_i[e] = sum_s w_s encT[e, s] on the Vector engine ----
            ot = soft_pool.tile([128, EC], f32, tag="ot")
            for c in range(EC):
                scr = wb_pool.tile([128, SW], f16, tag="scr",
                                   name=f"scr{c % 2}")
                nc.vector.scalar_tensor_tensor(
                    scr[:], et[:, c * SW:(c + 1) * SW], 1.0, psum_bc[:],
                    mult, mult, accum_out=ot[:, c:c + 1])
            nc.sync.dma_start(out=o_d[i, :, :], in_=ot[:])

        nc.sync.dma_start(out=z_d[:, :], in_=z_sb[:])
        nc.sync.dma_start(out=m_d[:, :], in_=m_sb[:])


def _build(nt):
    import concourse.bacc as bacc
    import concourse.tile as tile
    from concourse import mybir

    f32 = mybir.dt.float32
    f16 = mybir.dt.float16

    nc = bacc.Bacc("TRN2", target_bir_lowering=False, debug=False,
                   num_devices=N_CORES)
    encT3 = nc.dram_tensor("encT3", [nt, 128, EC, SW], f16,
                           kind="ExternalInput").ap()
    wencT3 = nc.dram_tensor("wencT3", [128, HT, EC, 128], f16,
                            kind="ExternalInput").ap()
    dprojT = nc.dram_tensor("dprojT", [128, HT * nt], f32,
                            kind="ExternalInput").ap()
    vTf = nc.dram_tensor("vTf", [128, HT], f32, kind="ExternalInput").ap()
    masks = nc.dram_tensor("masks", [1, nt * SW], f32, kind="ExternalInput").ap()
    o_d = nc.dram_tensor("o_slots", [nt, 128, EC], f32,
                         kind="ExternalOutput").ap()
    expu_d = nc.dram_tensor("expu_slots", [nt, SW], f16,
                            kind="ExternalOutput").ap()
    m_d = nc.dram_tensor("m_slots", [1, nt], f32, kind="ExternalOutput").ap()
    z_d = nc.dram_tensor("z_slots", [1, nt], f32, kind="ExternalOutput").ap()

    with tile.TileContext(nc) as tc:
        _emit(tc, mybir, nt, encT3, wencT3, dprojT, vTf, masks,
              o_d, expu_d, m_d, z_d)
    nc.compile()
    return nc


def get_compiled(nt):
    if nt not in _CACHE:
        _CACHE[nt] = _build(nt)
    return _CACHE[nt]


def plan_slots(sequence_length):
    """All (batch, s-tile) pairs with any valid position, dealt round-robin."""
    slots = []
    for b in range(B):
        for st in range(int(np.ceil(sequence_length[b] / SW))):
            slots.append((b, st))
    nt = max(1, (len(slots) + N_CORES - 1) // N_CORES)
    per_core = [slots[c::N_CORES] for c in range(N_CORES)]
    return per_core, nt


def make_in_maps(encoder_outputs, sequence_length, decoder_state, W, v):
    encoder_outputs = np.asarray(encoder_outputs, dtype=np.float32)
    sequence_length = np.asarray(sequence_length).astype(np.int64)
    decoder_state = np.asarray(decoder_state, dtype=np.float32)
    W = np.asarray(W, dtype=np.float32)
    v = np.asarray(v, dtype=np.float32)

    per_core, nt = plan_slots(sequence_length)

    # [p, t(h//128), c(e//128), h%128]
    wencT3 = np.ascontiguousarray(
        W[:, :E].reshape(HT, 128, EC, 128).transpose(3, 0, 2, 1)).astype(F16)
    vTf = np.ascontiguousarray(v.reshape(HT, 128).T).astype(np.float32)
    dproj = decoder_state @ W[:, E:].T                   # [B, H] f32
    # transposed copy for step A, tiled: [B, ST, 128(p), EC, SW]
    encT_all = np.ascontiguousarray(
        encoder_outputs.astype(F16).transpose(0, 2, 1)   # [B, E, S]
        .reshape(B, EC, 128, NST, SW)
        .transpose(0, 3, 2, 1, 4))                       # [B, ST, 128, EC, SW]
    svalid = np.arange(S).reshape(NST, SW)

    in_maps = []
    for cid in range(N_CORES):
        slots = per_core[cid]
        encT3 = np.zeros((nt, 128, EC, SW), F16)
        masks = np.full((nt, SW), MASK_VAL, np.float32)
        dpt = np.zeros((128, HT, nt), np.float32)
        for i, (b, st) in enumerate(slots):
            encT3[i] = encT_all[b, st]
            masks[i] = np.where(svalid[st] < sequence_length[b], 0.0, MASK_VAL)
            dpt[:, :, i] = dproj[b].reshape(HT, 128).T
        in_maps.append({
            "encT3": encT3,
            "wencT3": wencT3,
            "dprojT": np.ascontiguousarray(
                dpt.transpose(0, 1, 2).reshape(128, HT * nt)),
            "vTf": vTf,
            "masks": masks.reshape(1, nt * SW),
        })
    return in_maps, per_core, nt


def combine(results, per_core, sequence_length):
    """Host-side unshard: flash-style softmax combine across slots."""
    M = np.full(B, -np.inf)
    for cid in range(N_CORES):
        m = np.asarray(results[cid]["m_slots"], np.float64).reshape(-1)
        for i, (b, st) in enumerate(per_core[cid]):
            M[b] = max(M[b], m[i])
    Z = np.zeros(B, np.float64)
    out = np.zeros((B, E), np.float64)
    attn = np.zeros((B, S), np.float32)
    for cid in range(N_CORES):
        r = results[cid]
        o = np.asarray(r["o_slots"], np.float64)         # [nt, 128, EC]
        ex = np.asarray(r["expu_slots"], np.float32)     # [nt, SW] f16->f32
        z = np.asarray(r["z_slots"], np.float64).reshape(-1)
        m = np.asarray(r["m_slots"], np.float64).reshape(-1)
        for i, (b, st) in enumerate(per_core[cid]):
            w = np.exp(m[i] - M[b])
            Z[b] += w * z[i]
            out[b] += w * o[i].T.reshape(E)              # e = c*128 + p
            attn[b, st * SW:(st + 1) * SW] = ex[i] * np.float32(w)
    out = (out / Z[:, None]).astype(np.float32)
    attn = attn / Z[:, None].astype(np.float32)
    return out, attn.astype(np.float32)


def run(in_maps, nt, trace=False, **kw):
    from concourse.bass_utils import run_bass_kernel_spmd

    nc = get_compiled(nt)
    return run_bass_kernel_spmd(nc, in_maps, list(range(N_CORES)), trace=trace, **kw)


def kernel(encoder_outputs, sequence_length, decoder_state, W, v):
    in_maps, per_core, nt = make_in_maps(
        encoder_outputs, sequence_length, decoder_state, W, v)
    res = run(in_maps, nt)
    return combine(res.results, per_core, np.asarray(sequence_length))


# revision 15
# speedup vs baseline: 2.1035x; 1.0197x over previous
"""ConcatAttention (Bahdanau-style) Trainium2 kernel.

score = v^T tanh(W [h; s]);  softmax over masked sequence;  out = attn @ h.

Slot-sharded: the (batch, s-tile-of-512) pairs that contain any valid
(unmasked) position form "slots"; fully-masked tiles are skipped entirely
(their attention weights are exactly 0).  Slots are dealt round-robin across
the 8 NeuronCores for near-perfect balance; the tiny cross-slot softmax
combine (flash-attention style, using per-slot max m_i and sum z_i) is the
host-side unshard step.

Per slot, on device: enc_proj matmul in fp16 ([512,1024] x [1024,1024]) with
the decoder projection fused as the tanh bias; v-contraction on the Vector
engine in fp16 (2x mode) finished by a ones-vector matmul; slot-local
softmax numerator exp(l - m_i) in fp16; the rank-1 weighted sum
o_i[e] = sum_s w_s encT[e, s] runs on the Vector engine against the
already-resident transposed tile (no second encoder copy), with the weights
broadcast across partitions via a DRAM bounce.
"""

import numpy as np
import ml_dtypes

BF16 = ml_dtypes.bfloat16
F16 = np.float16

B, S, E, H = 32, 2048, 1024, 1024
D = 1024  # decoder dim (IN_FEATURES - E)
N_CORES = 8
MASK_VAL = -50000.0

SW = 512      # slot width (s positions per slot)
NST = S // SW  # 4 s-tiles per batch max
HT = 8        # h tiles of 128
EC = 8        # e (contraction) chunks of 128

_CACHE = {}


def _emit(tc, mybir, nt, encT3, wencT3, dprojT, vTf, masks, enc6last,
          o_d, expu_d, m_d, z_d, ol_d):
    nc = tc.nc
    f32 = mybir.dt.float32
    f16 = mybir.dt.float16
    Tanh = mybir.ActivationFunctionType.Tanh
    Exp = mybir.ActivationFunctionType.Exp
    Copy = mybir.ActivationFunctionType.Copy
    AX = mybir.AxisListType.X
    mult = mybir.AluOpType.mult
    add = mybir.AluOpType.add

    import concourse.bass as bass
    dmae = [nc.sync, nc.scalar, nc.gpsimd]

    from contextlib import ExitStack

    with ExitStack() as ctx:
        consts = ctx.enter_context(tc.tile_pool(name="consts", bufs=1))
        encT_pool = ctx.enter_context(tc.tile_pool(name="encT", bufs=5))
        hid_pool = ctx.enter_context(tc.tile_pool(name="hid", bufs=3))
        acc_pool = ctx.enter_context(tc.tile_pool(name="acc", bufs=2))
        soft_pool = ctx.enter_context(tc.tile_pool(name="soft", bufs=3))
        wb_pool = ctx.enter_context(tc.tile_pool(name="wb", bufs=2))
        pa_pool = ctx.enter_context(tc.tile_pool(name="pa", bufs=3, space="PSUM"))
        pl_pool = ctx.enter_context(tc.tile_pool(name="pl", bufs=2, space="PSUM"))
        pb_pool = ctx.enter_context(tc.tile_pool(name="pb", bufs=2, space="PSUM"))

        def load_et(i):
            et = encT_pool.tile([128, EC * SW], f16, tag="encT", name=f"et{i}")
            for q in range(4):
                dmae[q % 3].dma_start(
                    out=et[:, q * 2 * SW:(q + 1) * 2 * SW],
                    in_=encT3[i, :, q * 2:(q + 1) * 2, :],
                )
            return et

        # first slot's enc tile is the critical first dependency
        et0 = load_et(0)
        # step-A weights, t-major ([p, t*EC*128 + c*128 + h']), t=0 first so
        # the first t-group's weights land right after the first enc tile
        wenc_sb = consts.tile([128, HT * EC * 128], f16)
        for t in range(HT):
            dmae[(t + 1) % 3].dma_start(
                out=wenc_sb[:, t * 1024:(t + 1) * 1024],
                in_=wencT3[:, t, :, :],
            )
        vf_sb = consts.tile([128, HT], f32)
        nc.gpsimd.dma_start(out=vf_sb[:], in_=vTf[:, :])
        dproj = consts.tile([128, HT * nt], f32)
        nc.gpsimd.dma_start(out=dproj[:], in_=dprojT[:, :])
        ones16 = consts.tile([128, 1], f16)
        nc.vector.memset(ones16[:], 1.0)
        onesrow = consts.tile([1, 128], f16)
        nc.vector.memset(onesrow[:], 1.0)
        ones1 = consts.tile([1, 1], f16)
        nc.vector.memset(ones1[:], 1.0)
        z_sb = consts.tile([1, nt], f32)
        m_sb = consts.tile([1, nt], f32)
        mask_all = consts.tile([1, nt * SW], f32)
        nc.gpsimd.dma_start(out=mask_all[:], in_=masks[:, :])
        enc6l = consts.tile([128, (SW // 128) * E], f16)
        for half in range(2):
            dmae[half].dma_start(
                out=enc6l[:, half * 2 * E:(half + 1) * 2 * E],
                in_=enc6last[:, half * 2:(half + 1) * 2, :])

        for i in range(nt):
            # ---- step A: logits for this slot ----
            et = et0 if i == 0 else load_et(i)
            acc = acc_pool.tile([128, SW], f16, tag="acc")
            for t in range(HT):
                psum_a = pa_pool.tile([128, SW], f32, tag="pa")
                for c in range(EC):
                    nc.tensor.matmul(
                        psum_a[:],
                        lhsT=wenc_sb[:, t * 1024 + c * 128: t * 1024 + (c + 1) * 128],
                        rhs=et[:, c * SW:(c + 1) * SW],
                        start=(c == 0),
                        stop=(c == EC - 1),
                    )
                hid = hid_pool.tile([128, SW], f16, tag="hid")
                nc.scalar.activation(
                    hid[:], psum_a[:], Tanh,
                    bias=dproj[:, t * nt + i: t * nt + i + 1],
                )
                # v_t * tanh(...) as an ACT scaled copy; accumulate at DVE 2x
                if t == 0:
                    nc.scalar.activation(acc[:], hid[:], Copy,
                                         scale=vf_sb[:, 0:1])
                else:
                    sc = hid_pool.tile([128, SW], f16, tag="sc")
                    nc.scalar.activation(sc[:], hid[:], Copy,
                                         scale=vf_sb[:, t:t + 1])
                    nc.vector.tensor_add(acc[:], acc[:], sc[:])
            psum_l = pl_pool.tile([1, SW], f32, tag="pl")
            nc.tensor.matmul(psum_l[:], lhsT=ones16[:], rhs=acc[:],
                             start=True, stop=True)
            lg = soft_pool.tile([1, SW], f32, tag="lg")
            nc.vector.tensor_add(lg[:], psum_l[:],
                                 mask_all[:, i * SW:(i + 1) * SW])
            # ---- slot-local max and exp(l - m_i); z_i via ACT accumulator ----
            nc.vector.reduce_max(m_sb[:, i:i + 1], lg[:], axis=AX)
            negm = soft_pool.tile([1, 1], f32, tag="negm")
            nc.vector.tensor_scalar_mul(negm[:], m_sb[:, i:i + 1], -1.0)
            expu = soft_pool.tile([1, SW], f16, tag="expu")
            nc.scalar.activation(expu[:], lg[:], Exp, bias=negm[:],
                                 accum_out=z_sb[:, i:i + 1])
            nc.sync.dma_start(out=expu_d[i:i + 1, :], in_=expu[:])
            if i < nt - 1:
                # ---- broadcast weights across partitions via a k=1 matmul,
                #      then o_i[e] = sum_s w_s encT[e, s] on the Vector engine
                psum_bc = pb_pool.tile([128, SW], f32, tag="pb")
                nc.tensor.matmul(psum_bc[:], lhsT=onesrow[:], rhs=expu[:],
                                 start=True, stop=True)
                ot = soft_pool.tile([128, EC], f32, tag="ot")
                for c in range(EC):
                    scr = wb_pool.tile([128, SW], f16, tag="scr",
                                       name=f"scr{c % 2}")
                    nc.vector.scalar_tensor_tensor(
                        scr[:], et[:, c * SW:(c + 1) * SW], 1.0, psum_bc[:],
                        mult, mult, accum_out=ot[:, c:c + 1])
                nc.sync.dma_start(out=o_d[i, :, :], in_=ot[:])
            else:
                # ---- final slot: the PE is idle by now, so run the weighted
                #      sum there (transpose expu, then attn-stationary matmuls
                #      over the original-layout tile)
                psum_t = pb_pool.tile([128, SW // 128], f32, tag="pb",
                                      name="psum_t")
                for sc in range(SW // 128):
                    nc.tensor.matmul(
                        psum_t[:, sc:sc + 1],
                        lhsT=expu[:, sc * 128:(sc + 1) * 128],
                        rhs=ones1[:], start=True, stop=True)
                attnT = soft_pool.tile([128, SW // 128], f16, tag="attnT")
                nc.vector.tensor_copy(attnT[:], psum_t[:])
                psum_o = [pl_pool.tile([1, 512], f32, tag="pl", name=f"po{e_}")
                          for e_ in range(2)]
                for sc in range(SW // 128):
                    for eh in range(2):
                        nc.tensor.matmul(
                            psum_o[eh][:],
                            lhsT=attnT[:, sc:sc + 1],
                            rhs=enc6l[:, sc * E + eh * 512: sc * E + (eh + 1) * 512],
                            start=(sc == 0), stop=(sc == SW // 128 - 1))
                olast = soft_pool.tile([1, E], f32, tag="olast")
                for eh in range(2):
                    nc.scalar.activation(olast[:, eh * 512:(eh + 1) * 512],
                                         psum_o[eh][:], Copy)
                nc.sync.dma_start(out=ol_d[:, :], in_=olast[:])

        nc.sync.dma_start(out=z_d[:, :], in_=z_sb[:])
        nc.sync.dma_start(out=m_d[:, :], in_=m_sb[:])


def _build(nt):
    import concourse.bacc as bacc
    import concourse.tile as tile
    from concourse import mybir

    f32 = mybir.dt.float32
    f16 = mybir.dt.float16

    nc = bacc.Bacc("TRN2", target_bir_lowering=False, debug=False,
                   num_devices=N_CORES)
    encT3 = nc.dram_tensor("encT3", [nt, 128, EC, SW], f16,
                           kind="ExternalInput").ap()
    wencT3 = nc.dram_tensor("wencT3", [128, HT, EC, 128], f16,
                            kind="ExternalInput").ap()
    dprojT = nc.dram_tensor("dprojT", [128, HT * nt], f32,
                            kind="ExternalInput").ap()
    vTf = nc.dram_tensor("vTf", [128, HT], f32, kind="ExternalInput").ap()
    masks = nc.dram_tensor("masks", [1, nt * SW], f32, kind="ExternalInput").ap()
    enc6last = nc.dram_tensor("enc6last", [128, SW // 128, E], f16,
                              kind="ExternalInput").ap()
    o_d = nc.dram_tensor("o_slots", [nt, 128, EC], f32,
                         kind="ExternalOutput").ap()
    expu_d = nc.dram_tensor("expu_slots", [nt, SW], f16,
                            kind="ExternalOutput").ap()
    m_d = nc.dram_tensor("m_slots", [1, nt], f32, kind="ExternalOutput").ap()
    z_d = nc.dram_tensor("z_slots", [1, nt], f32, kind="ExternalOutput").ap()
    ol_d = nc.dram_tensor("o_last", [1, E], f32, kind="ExternalOutput").ap()

    with tile.TileContext(nc) as tc:
        _emit(tc, mybir, nt, encT3, wencT3, dprojT, vTf, masks, enc6last,
              o_d, expu_d, m_d, z_d, ol_d)
    nc.compile()
    return nc


def get_compiled(nt):
    if nt not in _CACHE:
        _CACHE[nt] = _build(nt)
    return _CACHE[nt]


def plan_slots(sequence_length):
    """All (batch, s-tile) pairs with any valid position, dealt round-robin."""
    slots = []
    for b in range(B):
        for st in range(int(np.ceil(sequence_length[b] / SW))):
            slots.append((b, st))
    nt = max(1, (len(slots) + N_CORES - 1) // N_CORES)
    per_core = [slots[c::N_CORES] for c in range(N_CORES)]
    return per_core, nt


def make_in_maps(encoder_outputs, sequence_length, decoder_state, W, v):
    encoder_outputs = np.asarray(encoder_outputs, dtype=np.float32)
    sequence_length = np.asarray(sequence_length).astype(np.int64)
    decoder_state = np.asarray(decoder_state, dtype=np.float32)
    W = np.asarray(W, dtype=np.float32)
    v = np.asarray(v, dtype=np.float32)

    per_core, nt = plan_slots(sequence_length)

    # [p, t(h//128), c(e//128), h%128]
    wencT3 = np.ascontiguousarray(
        W[:, :E].reshape(HT, 128, EC, 128).transpose(3, 0, 2, 1)).astype(F16)
    vTf = np.ascontiguousarray(v.reshape(HT, 128).T).astype(np.float32)
    dproj = decoder_state @ W[:, E:].T                   # [B, H] f32
    # transposed copy for step A, tiled: [B, ST, 128(p), EC, SW]
    encT_all = np.ascontiguousarray(
        encoder_outputs.astype(F16).transpose(0, 2, 1)   # [B, E, S]
        .reshape(B, EC, 128, NST, SW)
        .transpose(0, 3, 2, 1, 4))                       # [B, ST, 128, EC, SW]
    svalid = np.arange(S).reshape(NST, SW)

    in_maps = []
    for cid in range(N_CORES):
        slots = per_core[cid]
        encT3 = np.zeros((nt, 128, EC, SW), F16)
        masks = np.full((nt, SW), MASK_VAL, np.float32)
        dpt = np.zeros((128, HT, nt), np.float32)
        for i, (b, st) in enumerate(slots):
            encT3[i] = encT_all[b, st]
            masks[i] = np.where(svalid[st] < sequence_length[b], 0.0, MASK_VAL)
            dpt[:, :, i] = dproj[b].reshape(HT, 128).T
        if len(slots) == nt:
            lb, lst = slots[nt - 1]
            enc6last = np.ascontiguousarray(
                encoder_outputs[lb, lst * SW:(lst + 1) * SW, :]
                .reshape(SW // 128, 128, E).transpose(1, 0, 2)).astype(F16)
        else:
            enc6last = np.zeros((128, SW // 128, E), F16)
        in_maps.append({
            "encT3": encT3,
            "wencT3": wencT3,
            "dprojT": np.ascontiguousarray(
                dpt.transpose(0, 1, 2).reshape(128, HT * nt)),
            "vTf": vTf,
            "masks": masks.reshape(1, nt * SW),
            "enc6last": enc6last,
        })
    return in_maps, per_core, nt


def combine(results, per_core, sequence_length):
    """Host-side unshard: flash-style softmax combine across slots."""
    M = np.full(B, -np.inf)
    for cid in range(N_CORES):
        m = np.asarray(results[cid]["m_slots"], np.float64).reshape(-1)
        for i, (b, st) in enumerate(per_core[cid]):
            M[b] = max(M[b], m[i])
    Z = np.zeros(B, np.float64)
    out = np.zeros((B, E), np.float64)
    attn = np.zeros((B, S), np.float32)
    for cid in range(N_CORES):
        r = results[cid]
        o = np.asarray(r["o_slots"], np.float64)         # [nt, 128, EC]
        ex = np.asarray(r["expu_slots"], np.float32)     # [nt, SW] f16->f32
        z = np.asarray(r["z_slots"], np.float64).reshape(-1)
        m = np.asarray(r["m_slots"], np.float64).reshape(-1)
        nt = o.shape[0]
        ol = np.asarray(r["o_last"], np.float64).reshape(E)
        for i, (b, st) in enumerate(per_core[cid]):
            w = np.exp(m[i] - M[b])
            Z[b] += w * z[i]
            if i == nt - 1:
                out[b] += w * ol
            else:
                out[b] += w * o[i].T.reshape(E)          # e = c*128 + p
            attn[b, st * SW:(st + 1) * SW] = ex[i] * np.float32(w)
    out = (out / Z[:, None]).astype(np.float32)
    attn = attn / Z[:, None].astype(np.float32)
    return out, attn.astype(np.float32)


def run(in_maps, nt, trace=False, **kw):
    from concourse.bass_utils import run_bass_kernel_spmd

    nc = get_compiled(nt)
    return run_bass_kernel_spmd(nc, in_maps, list(range(N_CORES)), trace=trace, **kw)


def kernel(encoder_outputs, sequence_length, decoder_state, W, v):
    in_maps, per_core, nt = make_in_maps(
        encoder_outputs, sequence_length, decoder_state, W, v)
    res = run(in_maps, nt)
    return combine(res.results, per_core, np.asarray(sequence_length))


# revision 16
# speedup vs baseline: 2.1040x; 1.0002x over previous
"""ConcatAttention (Bahdanau-style) Trainium2 kernel.

score = v^T tanh(W [h; s]);  softmax over masked sequence;  out = attn @ h.

Slot-sharded: the (batch, s-tile-of-512) pairs that contain any valid
(unmasked) position form "slots"; fully-masked tiles are skipped entirely
(their attention weights are exactly 0).  Slots are dealt round-robin across
the 8 NeuronCores for near-perfect balance; the tiny cross-slot softmax
combine (flash-attention style, using per-slot max m_i and sum z_i) is the
host-side unshard step.

Per slot, on device: enc_proj matmul in fp16 ([512,1024] x [1024,1024]) with
the decoder projection fused as the tanh bias; v-contraction on the Vector
engine in fp16 (2x mode) finished by a ones-vector matmul; slot-local
softmax numerator exp(l - m_i) in fp16; the rank-1 weighted sum
o_i[e] = sum_s w_s encT[e, s] runs on the Vector engine against the
already-resident transposed tile (no second encoder copy), with the weights
broadcast across partitions via a DRAM bounce.
"""

import numpy as np
import ml_dtypes

BF16 = ml_dtypes.bfloat16
F16 = np.float16

B, S, E, H = 32, 2048, 1024, 1024
D = 1024  # decoder dim (IN_FEATURES - E)
N_CORES = 8
MASK_VAL = -50000.0

SW = 512      # slot width (s positions per slot)
NST = S // SW  # 4 s-tiles per batch max
HT = 8        # h tiles of 128
EC = 8        # e (contraction) chunks of 128

_CACHE = {}


def _emit(tc, mybir, nt, encT3, wencT3, dprojT, vTf, masks, enc6last,
          o_d, expu_d, m_d, z_d, ol_d):
    nc = tc.nc
    f32 = mybir.dt.float32
    f16 = mybir.dt.float16
    Tanh = mybir.ActivationFunctionType.Tanh
    Exp = mybir.ActivationFunctionType.Exp
    Copy = mybir.ActivationFunctionType.Copy
    AX = mybir.AxisListType.X
    mult = mybir.AluOpType.mult
    add = mybir.AluOpType.add

    import concourse.bass as bass
    dmae = [nc.sync, nc.scalar, nc.gpsimd]

    from contextlib import ExitStack

    with ExitStack() as ctx:
        consts = ctx.enter_context(tc.tile_pool(name="consts", bufs=1))
        encT_pool = ctx.enter_context(tc.tile_pool(name="encT", bufs=5))
        hid_pool = ctx.enter_context(tc.tile_pool(name="hid", bufs=3))
        acc_pool = ctx.enter_context(tc.tile_pool(name="acc", bufs=2))
        soft_pool = ctx.enter_context(tc.tile_pool(name="soft", bufs=3))
        wb_pool = ctx.enter_context(tc.tile_pool(name="wb", bufs=2))
        pa_pool = ctx.enter_context(tc.tile_pool(name="pa", bufs=3, space="PSUM"))
        pl_pool = ctx.enter_context(tc.tile_pool(name="pl", bufs=2, space="PSUM"))
        pb_pool = ctx.enter_context(tc.tile_pool(name="pb", bufs=2, space="PSUM"))

        def load_et(i):
            et = encT_pool.tile([128, EC * SW], f16, tag="encT", name=f"et{i}")
            for q in range(4):
                dmae[q % 3].dma_start(
                    out=et[:, q * 2 * SW:(q + 1) * 2 * SW],
                    in_=encT3[i, :, q * 2:(q + 1) * 2, :],
                )
            return et

        # first slot's enc tile + t=0 weights are the critical first deps
        wenc_sb = consts.tile([128, HT * EC * 128], f16)
        nc.sync.dma_start(out=wenc_sb[:, 0:1024], in_=wencT3[:, 0, :, :])
        et0 = load_et(0)
        for t in range(1, HT):
            dmae[t % 3].dma_start(
                out=wenc_sb[:, t * 1024:(t + 1) * 1024],
                in_=wencT3[:, t, :, :],
            )
        vf_sb = consts.tile([128, HT], f32)
        nc.gpsimd.dma_start(out=vf_sb[:], in_=vTf[:, :])
        dproj = consts.tile([128, HT * nt], f32)
        nc.gpsimd.dma_start(out=dproj[:], in_=dprojT[:, :])
        ones16 = consts.tile([128, 1], f16)
        nc.vector.memset(ones16[:], 1.0)
        onesrow = consts.tile([1, 128], f16)
        nc.vector.memset(onesrow[:], 1.0)
        ones1 = consts.tile([1, 1], f16)
        nc.vector.memset(ones1[:], 1.0)
        z_sb = consts.tile([1, nt], f32)
        m_sb = consts.tile([1, nt], f32)
        mask_all = consts.tile([1, nt * SW], f32)
        nc.gpsimd.dma_start(out=mask_all[:], in_=masks[:, :])
        enc6l = consts.tile([128, (SW // 128) * E], f16)
        for half in range(2):
            dmae[half].dma_start(
                out=enc6l[:, half * 2 * E:(half + 1) * 2 * E],
                in_=enc6last[:, half * 2:(half + 1) * 2, :])

        for i in range(nt):
            # ---- step A: logits for this slot ----
            et = et0 if i == 0 else load_et(i)
            acc = acc_pool.tile([128, SW], f16, tag="acc")
            for t in range(HT):
                psum_a = pa_pool.tile([128, SW], f32, tag="pa")
                for c in range(EC):
                    nc.tensor.matmul(
                        psum_a[:],
                        lhsT=wenc_sb[:, t * 1024 + c * 128: t * 1024 + (c + 1) * 128],
                        rhs=et[:, c * SW:(c + 1) * SW],
                        start=(c == 0),
                        stop=(c == EC - 1),
                    )
                hid = hid_pool.tile([128, SW], f16, tag="hid")
                nc.scalar.activation(
                    hid[:], psum_a[:], Tanh,
                    bias=dproj[:, t * nt + i: t * nt + i + 1],
                )
                # v_t * tanh(...) as an ACT scaled copy; accumulate at DVE 2x
                if t == 0:
                    nc.scalar.activation(acc[:], hid[:], Copy,
                                         scale=vf_sb[:, 0:1])
                else:
                    sc = hid_pool.tile([128, SW], f16, tag="sc")
                    nc.scalar.activation(sc[:], hid[:], Copy,
                                         scale=vf_sb[:, t:t + 1])
                    nc.vector.tensor_add(acc[:], acc[:], sc[:])
            psum_l = pl_pool.tile([1, SW], f32, tag="pl")
            nc.tensor.matmul(psum_l[:], lhsT=ones16[:], rhs=acc[:],
                             start=True, stop=True)
            lg = soft_pool.tile([1, SW], f32, tag="lg")
            nc.vector.tensor_add(lg[:], psum_l[:],
                                 mask_all[:, i * SW:(i + 1) * SW])
            # ---- slot-local max and exp(l - m_i); z_i via ACT accumulator ----
            nc.vector.reduce_max(m_sb[:, i:i + 1], lg[:], axis=AX)
            negm = soft_pool.tile([1, 1], f32, tag="negm")
            nc.vector.tensor_scalar_mul(negm[:], m_sb[:, i:i + 1], -1.0)
            expu = soft_pool.tile([1, SW], f16, tag="expu")
            nc.scalar.activation(expu[:], lg[:], Exp, bias=negm[:],
                                 accum_out=z_sb[:, i:i + 1])
            nc.sync.dma_start(out=expu_d[i:i + 1, :], in_=expu[:])
            if i < nt - 1:
                # ---- broadcast weights across partitions via a k=1 matmul,
                #      then o_i[e] = sum_s w_s encT[e, s] on the Vector engine
                psum_bc = pb_pool.tile([128, SW], f32, tag="pb")
                nc.tensor.matmul(psum_bc[:], lhsT=onesrow[:], rhs=expu[:],
                                 start=True, stop=True)
                ot = soft_pool.tile([128, EC], f32, tag="ot")
                for c in range(EC):
                    scr = wb_pool.tile([128, SW], f16, tag="scr",
                                       name=f"scr{c % 2}")
                    nc.vector.scalar_tensor_tensor(
                        scr[:], et[:, c * SW:(c + 1) * SW], 1.0, psum_bc[:],
                        mult, mult, accum_out=ot[:, c:c + 1])
                nc.sync.dma_start(out=o_d[i, :, :], in_=ot[:])
            else:
                # ---- final slot: the PE is idle by now, so run the weighted
                #      sum there (transpose expu, then attn-stationary matmuls
                #      over the original-layout tile)
                psum_t = pb_pool.tile([128, SW // 128], f32, tag="pb",
                                      name="psum_t")
                for sc in range(SW // 128):
                    nc.tensor.matmul(
                        psum_t[:, sc:sc + 1],
                        lhsT=expu[:, sc * 128:(sc + 1) * 128],
                        rhs=ones1[:], start=True, stop=True)
                attnT = soft_pool.tile([128, SW // 128], f16, tag="attnT")
                nc.vector.tensor_copy(attnT[:], psum_t[:])
                psum_o = [pl_pool.tile([1, 512], f32, tag="pl", name=f"po{e_}")
                          for e_ in range(2)]
                for sc in range(SW // 128):
                    for eh in range(2):
                        nc.tensor.matmul(
                            psum_o[eh][:],
                            lhsT=attnT[:, sc:sc + 1],
                            rhs=enc6l[:, sc * E + eh * 512: sc * E + (eh + 1) * 512],
                            start=(sc == 0), stop=(sc == SW // 128 - 1))
                olast = soft_pool.tile([1, E], f32, tag="olast")
                for eh in range(2):
                    nc.scalar.activation(olast[:, eh * 512:(eh + 1) * 512],
                                         psum_o[eh][:], Copy)
                nc.sync.dma_start(out=ol_d[:, :], in_=olast[:])

        nc.sync.dma_start(out=z_d[:, :], in_=z_sb[:])
        nc.sync.dma_start(out=m_d[:, :], in_=m_sb[:])


def _build(nt):
    import concourse.bacc as bacc
    import concourse.tile as tile
    from concourse import mybir

    f32 = mybir.dt.float32
    f16 = mybir.dt.float16

    nc = bacc.Bacc("TRN2", target_bir_lowering=False, debug=False,
                   num_devices=N_CORES)
    encT3 = nc.dram_tensor("encT3", [nt, 128, EC, SW], f16,
                           kind="ExternalInput").ap()
    wencT3 = nc.dram_tensor("wencT3", [128, HT, EC, 128], f16,
                            kind="ExternalInput").ap()
    dprojT = nc.dram_tensor("dprojT", [128, HT * nt], f32,
                            kind="ExternalInput").ap()
    vTf = nc.dram_tensor("vTf", [128, HT], f32, kind="ExternalInput").ap()
    masks = nc.dram_tensor("masks", [1, nt * SW], f32, kind="ExternalInput").ap()
    enc6last = nc.dram_tensor("enc6last", [128, SW // 128, E], f16,
                              kind="ExternalInput").ap()
    o_d = nc.dram_tensor("o_slots", [nt, 128, EC], f32,
                         kind="ExternalOutput").ap()
    expu_d = nc.dram_tensor("expu_slots", [nt, SW], f16,
                            kind="ExternalOutput").ap()
    m_d = nc.dram_tensor("m_slots", [1, nt], f32, kind="ExternalOutput").ap()
    z_d = nc.dram_tensor("z_slots", [1, nt], f32, kind="ExternalOutput").ap()
    ol_d = nc.dram_tensor("o_last", [1, E], f32, kind="ExternalOutput").ap()

    with tile.TileContext(nc) as tc:
        _emit(tc, mybir, nt, encT3, wencT3, dprojT, vTf, masks, enc6last,
              o_d, expu_d, m_d, z_d, ol_d)
    nc.compile()
    return nc


def get_compiled(nt):
    if nt not in _CACHE:
        _CACHE[nt] = _build(nt)
    return _CACHE[nt]


def plan_slots(sequence_length):
    """All (batch, s-tile) pairs with any valid position, dealt round-robin."""
    slots = []
    for b in range(B):
        for st in range(int(np.ceil(sequence_length[b] / SW))):
            slots.append((b, st))
    nt = max(1, (len(slots) + N_CORES - 1) // N_CORES)
    per_core = [slots[c::N_CORES] for c in range(N_CORES)]
    return per_core, nt


def make_in_maps(encoder_outputs, sequence_length, decoder_state, W, v):
    encoder_outputs = np.asarray(encoder_outputs, dtype=np.float32)
    sequence_length = np.asarray(sequence_length).astype(np.int64)
    decoder_state = np.asarray(decoder_state, dtype=np.float32)
    W = np.asarray(W, dtype=np.float32)
    v = np.asarray(v, dtype=np.float32)

    per_core, nt = plan_slots(sequence_length)

    # [p, t(h//128), c(e//128), h%128]
    wencT3 = np.ascontiguousarray(
        W[:, :E].reshape(HT, 128, EC, 128).transpose(3, 0, 2, 1)).astype(F16)
    vTf = np.ascontiguousarray(v.reshape(HT, 128).T).astype(np.float32)
    dproj = decoder_state @ W[:, E:].T                   # [B, H] f32
    # transposed copy for step A, tiled: [B, ST, 128(p), EC, SW]
    encT_all = np.ascontiguousarray(
        encoder_outputs.astype(F16).transpose(0, 2, 1)   # [B, E, S]
        .reshape(B, EC, 128, NST, SW)
        .transpose(0, 3, 2, 1, 4))                       # [B, ST, 128, EC, SW]
    svalid = np.arange(S).reshape(NST, SW)

    in_maps = []
    for cid in range(N_CORES):
        slots = per_core[cid]
        encT3 = np.zeros((nt, 128, EC, SW), F16)
        masks = np.full((nt, SW), MASK_VAL, np.float32)
        dpt = np.zeros((128, HT, nt), np.float32)
        for i, (b, st) in enumerate(slots):
            encT3[i] = encT_all[b, st]
            masks[i] = np.where(svalid[st] < sequence_length[b], 0.0, MASK_VAL)
            dpt[:, :, i] = dproj[b].reshape(HT, 128).T
        if len(slots) == nt:
            lb, lst = slots[nt - 1]
            enc6last = np.ascontiguousarray(
                encoder_outputs[lb, lst * SW:(lst + 1) * SW, :]
                .reshape(SW // 128, 128, E).transpose(1, 0, 2)).astype(F16)
        else:
            enc6last = np.zeros((128, SW // 128, E), F16)
        in_maps.append({
            "encT3": encT3,
            "wencT3": wencT3,
            "dprojT": np.ascontiguousarray(
                dpt.transpose(0, 1, 2).reshape(128, HT * nt)),
            "vTf": vTf,
            "masks": masks.reshape(1, nt * SW),
            "enc6last": enc6last,
        })
    return in_maps, per_core, nt


def combine(results, per_core, sequence_length):
    """Host-side unshard: flash-style softmax combine across slots."""
    M = np.full(B, -np.inf)
    for cid in range(N_CORES):
        m = np.asarray(results[cid]["m_slots"], np.float64).reshape(-1)
        for i, (b, st) in enumerate(per_core[cid]):
            M[b] = max(M[b], m[i])
    Z = np.zeros(B, np.float64)
    out = np.zeros((B, E), np.float64)
    attn = np.zeros((B, S), np.float32)
    for cid in range(N_CORES):
        r = results[cid]
        o = np.asarray(r["o_slots"], np.float64)         # [nt, 128, EC]
        ex = np.asarray(r["expu_slots"], np.float32)     # [nt, SW] f16->f32
        z = np.asarray(r["z_slots"], np.float64).reshape(-1)
        m = np.asarray(r["m_slots"], np.float64).reshape(-1)
        nt = o.shape[0]
        ol = np.asarray(r["o_last"], np.float64).reshape(E)
        for i, (b, st) in enumerate(per_core[cid]):
            w = np.exp(m[i] - M[b])
            Z[b] += w * z[i]
            if i == nt - 1:
                out[b] += w * ol
            else:
                out[b] += w * o[i].T.reshape(E)          # e = c*128 + p
            attn[b, st * SW:(st + 1) * SW] = ex[i] * np.float32(w)
    out = (out / Z[:, None]).astype(np.float32)
    attn = attn / Z[:, None].astype(np.float32)
    return out, attn.astype(np.float32)


def run(in_maps, nt, trace=False, **kw):
    from concourse.bass_utils import run_bass_kernel_spmd

    nc = get_compiled(nt)
    return run_bass_kernel_spmd(nc, in_maps, list(range(N_CORES)), trace=trace, **kw)


def kernel(encoder_outputs, sequence_length, decoder_state, W, v):
    in_maps, per_core, nt = make_in_maps(
        encoder_outputs, sequence_length, decoder_state, W, v)
    res = run(in_maps, nt)
    return combine(res.results, per_core, np.asarray(sequence_length))


# revision 27
# speedup vs baseline: 2.4253x; 1.1527x over previous
"""ConcatAttention (Bahdanau-style) Trainium2 kernel.

score = v^T tanh(W [h; s]);  softmax over masked sequence;  out = attn @ h.

Slot-sharded: the (batch, s-tile-of-512) pairs that contain any valid
(unmasked) position form "slots"; fully-masked tiles are skipped entirely
(their attention weights are exactly 0).  Slots are dealt round-robin across
the 8 NeuronCores for near-perfect balance; the tiny cross-slot softmax
combine (flash-attention style, using per-slot max m_i and sum z_i) is the
host-side unshard step.

Per slot, on device: enc_proj matmul in fp16 ([512,1024] x [1024,1024]) with
the decoder projection fused as the tanh bias; v-contraction on the Vector
engine in fp16 (2x mode) finished by a ones-vector matmul; slot-local
softmax numerator exp(l - m_i) in fp16; the rank-1 weighted sum
o_i[e] = sum_s w_s encT[e, s] runs on the Vector engine against the
already-resident transposed tile (no second encoder copy), with the weights
broadcast across partitions via a k=1 ones matmul into PSUM.
"""

import numpy as np
import ml_dtypes

BF16 = ml_dtypes.bfloat16
F16 = np.float16

B, S, E, H = 32, 2048, 1024, 1024
D = 1024  # decoder dim (IN_FEATURES - E)
N_CORES = 8
MASK_VAL = -50000.0

SW = 512      # slot width (s positions per slot)
NST = S // SW  # 4 s-tiles per batch max
HT = 8        # h tiles of 128
EC = 8        # e (contraction) chunks of 128

_CACHE = {}


def _emit(tc, mybir, pattern, encT3, wencT3, dprojT, vTf, masks, enc6last,
          o_d, expu_d, m_d, z_d, ol_d):
    nt = len(pattern)
    nc = tc.nc
    f32 = mybir.dt.float32
    f16 = mybir.dt.float16
    Tanh = mybir.ActivationFunctionType.Tanh
    Exp = mybir.ActivationFunctionType.Exp
    Copy = mybir.ActivationFunctionType.Copy
    AX = mybir.AxisListType.X
    mult = mybir.AluOpType.mult
    add = mybir.AluOpType.add

    import concourse.bass as bass
    dmae = [nc.sync, nc.scalar, nc.gpsimd]

    from contextlib import ExitStack

    with ExitStack() as ctx:
        consts = ctx.enter_context(tc.tile_pool(name="consts", bufs=1))
        encT_pool = ctx.enter_context(tc.tile_pool(name="encT", bufs=6))
        hid_pool = ctx.enter_context(tc.tile_pool(name="hid", bufs=3))
        acc_pool = ctx.enter_context(tc.tile_pool(name="acc", bufs=2))
        soft_pool = ctx.enter_context(tc.tile_pool(name="soft", bufs=3))
        wb_pool = ctx.enter_context(tc.tile_pool(name="wb", bufs=2))
        pa_pool = ctx.enter_context(tc.tile_pool(name="pa", bufs=4, space="PSUM"))
        pl_pool = ctx.enter_context(tc.tile_pool(name="pl", bufs=2, space="PSUM"))
        pb_pool = ctx.enter_context(tc.tile_pool(name="pb", bufs=2, space="PSUM"))

        def load_et(i):
            w = pattern[i] * 128
            et = encT_pool.tile([128, EC * w], f16, tag="encT", name=f"et{i}")
            # first slot: one DMA per c-chunk so the c-loop can start on the
            # first chunk; later slots: coarser halves
            ng = EC if i == 0 else 4
            step = EC // ng
            for q in range(ng):
                dmae[q % 3].dma_start(
                    out=et[:, q * step * w:(q + 1) * step * w],
                    in_=encT3[i, :, q * step:(q + 1) * step, :w],
                )
            return et

        # first slot's enc tile + t=0 weights are the critical first deps
        wenc_sb = consts.tile([128, HT * EC * 128], f16)
        nc.sync.dma_start(out=wenc_sb[:, 0:1024], in_=wencT3[:, 0, :, :])
        et0 = load_et(0)
        for t in range(1, HT):
            dmae[t % 3].dma_start(
                out=wenc_sb[:, t * 1024:(t + 1) * 1024],
                in_=wencT3[:, t, :, :],
            )
        vf_sb = consts.tile([128, HT], f32)
        nc.gpsimd.dma_start(out=vf_sb[:], in_=vTf[:, :])
        dproj = consts.tile([128, HT * nt], f32)
        nc.gpsimd.dma_start(out=dproj[:], in_=dprojT[:, :])
        ones16 = consts.tile([128, 1], f16)
        nc.vector.memset(ones16[:], 1.0)
        onesrow = consts.tile([1, 128], f16)
        nc.vector.memset(onesrow[:], 1.0)
        ones1 = consts.tile([1, 1], f16)
        nc.vector.memset(ones1[:], 1.0)
        z_sb = consts.tile([1, nt], f32)
        m_sb = consts.tile([1, nt], f32)
        mask_all = consts.tile([1, nt * SW], f32)
        nc.gpsimd.dma_start(out=mask_all[:], in_=masks[:, :])
        wql = pattern[nt - 1]
        enc6l = consts.tile([128, wql * E], f16)
        for qq in range(wql):
            dmae[qq % 3].dma_start(
                out=enc6l[:, qq * E:(qq + 1) * E],
                in_=enc6last[:, qq, :])

        for i in range(nt):
            w = pattern[i] * 128
            # ---- step A: logits for this slot ----
            et = et0 if i == 0 else load_et(i)
            acc = acc_pool.tile([128, w], f16, tag="acc")
            for t in range(HT):
                psum_a = pa_pool.tile([128, w], f32, tag="pa")
                for c in range(EC):
                    nc.tensor.matmul(
                        psum_a[:],
                        lhsT=wenc_sb[:, t * 1024 + c * 128: t * 1024 + (c + 1) * 128],
                        rhs=et[:, c * w:(c + 1) * w],
                        start=(c == 0),
                        stop=(c == EC - 1),
                    )
                hid = hid_pool.tile([128, w], f16, tag="hid")
                nc.scalar.activation(
                    hid[:], psum_a[:], Tanh,
                    bias=dproj[:, t * nt + i: t * nt + i + 1],
                )
                # v_t * tanh(...) as an ACT scaled copy; accumulate at DVE 2x
                if t == 0:
                    nc.scalar.activation(acc[:], hid[:], Copy,
                                         scale=vf_sb[:, 0:1])
                else:
                    sc = hid_pool.tile([128, w], f16, tag="sc")
                    nc.scalar.activation(sc[:], hid[:], Copy,
                                         scale=vf_sb[:, t:t + 1])
                    nc.vector.tensor_add(acc[:], acc[:], sc[:])
            psum_l = pl_pool.tile([1, w], f32, tag="pl")
            nc.tensor.matmul(psum_l[:], lhsT=ones16[:], rhs=acc[:],
                             start=True, stop=True)
            lg = soft_pool.tile([1, w], f32, tag="lg")
            nc.vector.tensor_add(lg[:], psum_l[:],
                                 mask_all[:, i * 512:i * 512 + w])
            # ---- slot-local max and exp(l - m_i); z_i via ACT accumulator ----
            nc.vector.reduce_max(m_sb[:, i:i + 1], lg[:], axis=AX)
            negm = soft_pool.tile([1, 1], f32, tag="negm")
            nc.vector.tensor_scalar_mul(negm[:], m_sb[:, i:i + 1], -1.0)
            expu = soft_pool.tile([1, w], f16, tag="expu")
            nc.scalar.activation(expu[:], lg[:], Exp, bias=negm[:],
                                 accum_out=z_sb[:, i:i + 1])
            nc.sync.dma_start(out=expu_d[i:i + 1, :w], in_=expu[:])
            if i < nt - 1:
                # ---- broadcast weights across partitions via a k=1 matmul,
                #      then o_i[e] = sum_s w_s encT[e, s] on the Vector engine
                psum_bc = pb_pool.tile([128, w], f32, tag="pb")
                nc.tensor.matmul(psum_bc[:], lhsT=onesrow[:], rhs=expu[:],
                                 start=True, stop=True)
                ot = soft_pool.tile([128, EC], f32, tag="ot")
                for c in range(EC):
                    scr = wb_pool.tile([128, w], f16, tag="scr",
                                       name=f"scr{c % 2}")
                    nc.vector.scalar_tensor_tensor(
                        scr[:], et[:, c * w:(c + 1) * w], 1.0, psum_bc[:],
                        mult, mult, accum_out=ot[:, c:c + 1])
                nc.sync.dma_start(out=o_d[i, :, :], in_=ot[:])
            else:
                # ---- final slot: the PE is idle by now, so run the weighted
                #      sum there (transpose expu, then attn-stationary matmuls
                #      over the original-layout tile)
                psum_t = pb_pool.tile([128, w // 128], f32, tag="pb",
                                      name="psum_t")
                for sc in range(w // 128):
                    nc.tensor.matmul(
                        psum_t[:, sc:sc + 1],
                        lhsT=expu[:, sc * 128:(sc + 1) * 128],
                        rhs=ones1[:], start=True, stop=True)
                attnT = soft_pool.tile([128, w // 128], f16, tag="attnT")
                nc.vector.tensor_copy(attnT[:], psum_t[:])
                psum_o = [pl_pool.tile([1, 512], f32, tag="pl", name=f"po{e_}")
                          for e_ in range(2)]
                for sc in range(w // 128):
                    for eh in range(2):
                        nc.tensor.matmul(
                            psum_o[eh][:],
                            lhsT=attnT[:, sc:sc + 1],
                            rhs=enc6l[:, sc * E + eh * 512: sc * E + (eh + 1) * 512],
                            start=(sc == 0), stop=(sc == w // 128 - 1))
                olast = soft_pool.tile([1, E], f32, tag="olast")
                for eh in range(2):
                    nc.scalar.activation(olast[:, eh * 512:(eh + 1) * 512],
                                         psum_o[eh][:], Copy)
                nc.sync.dma_start(out=ol_d[:, :], in_=olast[:])

        nc.sync.dma_start(out=z_d[:, :], in_=z_sb[:])
        nc.sync.dma_start(out=m_d[:, :], in_=m_sb[:])


def _build(pattern):
    import concourse.bacc as bacc
    import concourse.tile as tile
    from concourse import mybir

    f32 = mybir.dt.float32
    f16 = mybir.dt.float16
    nt = len(pattern)

    nc = bacc.Bacc("TRN2", target_bir_lowering=False, debug=False,
                   num_devices=N_CORES)
    encT3 = nc.dram_tensor("encT3", [nt, 128, EC, SW], f16,
                           kind="ExternalInput").ap()
    wencT3 = nc.dram_tensor("wencT3", [128, HT, EC, 128], f16,
                            kind="ExternalInput").ap()
    dprojT = nc.dram_tensor("dprojT", [128, HT * nt], f32,
                            kind="ExternalInput").ap()
    vTf = nc.dram_tensor("vTf", [128, HT], f32, kind="ExternalInput").ap()
    masks = nc.dram_tensor("masks", [1, nt * SW], f32, kind="ExternalInput").ap()
    enc6last = nc.dram_tensor("enc6last", [128, pattern[nt - 1], E], f16,
                              kind="ExternalInput").ap()
    o_d = nc.dram_tensor("o_slots", [nt, 128, EC], f32,
                         kind="ExternalOutput").ap()
    expu_d = nc.dram_tensor("expu_slots", [nt, SW], f16,
                            kind="ExternalOutput").ap()
    m_d = nc.dram_tensor("m_slots", [1, nt], f32, kind="ExternalOutput").ap()
    z_d = nc.dram_tensor("z_slots", [1, nt], f32, kind="ExternalOutput").ap()
    ol_d = nc.dram_tensor("o_last", [1, E], f32, kind="ExternalOutput").ap()

    with tile.TileContext(nc) as tc:
        _emit(tc, mybir, pattern, encT3, wencT3, dprojT, vTf, masks, enc6last,
              o_d, expu_d, m_d, z_d, ol_d)
    nc.compile()
    return nc


def get_compiled(pattern):
    key = tuple(pattern)
    if key not in _CACHE:
        _CACHE[key] = _build(list(key))
    return _CACHE[key]


def plan_slots(sequence_length):
    """Pack per-batch valid ranges (128-quanta granularity) into mixed-width
    slots: LPT-deal whole batches across cores, cut into chunks of <= 4
    quanta, sort desc; the shared compile-time width pattern is the
    positionwise max.  Returns per_core lists of (b, q0, wq_valid) and the
    pattern (widths in quanta)."""
    q = np.maximum(1, np.ceil(np.asarray(sequence_length) / 128)).astype(int)
    order = np.argsort(-q)
    loads = [0] * N_CORES
    segs = [[] for _ in range(N_CORES)]
    for b in order:
        c = int(np.argmin(loads))
        loads[c] += int(q[b])
        segs[c].append(int(b))
    per_core = []
    for c in range(N_CORES):
        sl = []
        for b in segs[c]:
            q0, rem = 0, int(q[b])
            while rem > 0:
                t = min(4, rem)
                sl.append((b, q0, t))
                q0 += t
                rem -= t
        sl.sort(key=lambda x: -x[2])
        per_core.append(sl)
    k = max(1, max(len(s) for s in per_core))
    pattern = [max((s[j][2] if j < len(s) else 1) for s in per_core)
               for j in range(k)]
    return per_core, pattern


def make_in_maps(encoder_outputs, sequence_length, decoder_state, W, v):
    encoder_outputs = np.asarray(encoder_outputs, dtype=np.float32)
    sequence_length = np.asarray(sequence_length).astype(np.int64)
    decoder_state = np.asarray(decoder_state, dtype=np.float32)
    W = np.asarray(W, dtype=np.float32)
    v = np.asarray(v, dtype=np.float32)

    per_core, pattern = plan_slots(sequence_length)
    nt = len(pattern)
    wql = pattern[nt - 1]

    # [p, t(h//128), c(e//128), h%128]
    wencT3 = np.ascontiguousarray(
        W[:, :E].reshape(HT, 128, EC, 128).transpose(3, 0, 2, 1)).astype(F16)
    vTf = np.ascontiguousarray(v.reshape(HT, 128).T).astype(np.float32)
    dproj = decoder_state @ W[:, E:].T                   # [B, H] f32
    # transposed copy for step A: [B, 128(p), EC, S]
    encTp = np.ascontiguousarray(
        encoder_outputs.astype(F16).transpose(0, 2, 1)   # [B, E, S]
        .reshape(B, EC, 128, S)
        .transpose(0, 2, 1, 3))                          # [B, 128, EC, S]
    sidx = np.arange(S)

    in_maps = []
    for cid in range(N_CORES):
        slots = per_core[cid]
        encT3 = np.zeros((nt, 128, EC, SW), F16)
        masks = np.full((nt, SW), MASK_VAL, np.float32)
        dpt = np.zeros((128, HT, nt), np.float32)
        for i, (b, q0, wq) in enumerate(slots):
            s0, wv = q0 * 128, wq * 128
            encT3[i, :, :, :wv] = encTp[b, :, :, s0:s0 + wv]
            masks[i, :wv] = np.where(sidx[s0:s0 + wv] < sequence_length[b],
                                     0.0, MASK_VAL)
            dpt[:, :, i] = dproj[b].reshape(HT, 128).T
        enc6last = np.zeros((128, wql, E), F16)
        if len(slots) == nt:
            lb, lq0, lwq = slots[nt - 1]
            seg = encoder_outputs[lb, lq0 * 128:(lq0 + lwq) * 128, :]
            enc6last[:, :lwq, :] = seg.reshape(lwq, 128, E).transpose(1, 0, 2)
        in_maps.append({
            "encT3": encT3,
            "wencT3": wencT3,
            "dprojT": np.ascontiguousarray(
                dpt.transpose(0, 1, 2).reshape(128, HT * nt)),
            "vTf": vTf,
            "masks": masks.reshape(1, nt * SW),
            "enc6last": np.ascontiguousarray(enc6last).astype(F16),
        })
    return in_maps, per_core, pattern


def combine(results, per_core, sequence_length):
    """Host-side unshard: flash-style softmax combine across slots."""
    M = np.full(B, -np.inf)
    for cid in range(N_CORES):
        m = np.asarray(results[cid]["m_slots"], np.float64).reshape(-1)
        for i, (b, q0, wq) in enumerate(per_core[cid]):
            M[b] = max(M[b], m[i])
    Z = np.zeros(B, np.float64)
    out = np.zeros((B, E), np.float64)
    attn = np.zeros((B, S), np.float32)
    for cid in range(N_CORES):
        r = results[cid]
        o = np.asarray(r["o_slots"], np.float64)         # [nt, 128, EC]
        ex = np.asarray(r["expu_slots"], np.float32)     # [nt, SW] f16->f32
        z = np.asarray(r["z_slots"], np.float64).reshape(-1)
        m = np.asarray(r["m_slots"], np.float64).reshape(-1)
        nt = o.shape[0]
        ol = np.asarray(r["o_last"], np.float64).reshape(E)
        for i, (b, q0, wq) in enumerate(per_core[cid]):
            w = np.exp(m[i] - M[b])
            Z[b] += w * z[i]
            if i == nt - 1:
                out[b] += w * ol
            else:
                out[b] += w * o[i].T.reshape(E)          # e = c*128 + p
            s0, wv = q0 * 128, wq * 128
            attn[b, s0:s0 + wv] = ex[i, :wv] * np.float32(w)
    out = (out / Z[:, None]).astype(np.float32)
    attn = attn / Z[:, None].astype(np.float32)
    return out, attn.astype(np.float32)


def run(in_maps, pattern, trace=False, **kw):
    import os

    from concourse.bass_utils import run_bass_kernel_spmd

    if not trace:
        # this image lacks antenv.axon_hooks; a stray BASS_TRACE env var
        # would crash the axon trace path inside run_bass_kernel_spmd
        os.environ["BASS_NEVER_TRACE"] = "1"
    else:
        os.environ.pop("BASS_NEVER_TRACE", None)
    nc = get_compiled(pattern)
    return run_bass_kernel_spmd(nc, in_maps, list(range(N_CORES)), trace=trace, **kw)


def kernel(encoder_outputs, sequence_length, decoder_state, W, v):
    in_maps, per_core, pattern = make_in_maps(
        encoder_outputs, sequence_length, decoder_state, W, v)
    res = run(in_maps, pattern)
    return combine(res.results, per_core, np.asarray(sequence_length))
